# revision 1
# baseline (speedup 1.0000x reference)
"""DBRX MoE experts kernel for Trainium2 (8 NeuronCores).

Strategy:
  - Router (logits -> softmax -> top-2 -> renormalize) computed on host in numpy
    (0.01% of FLOPs); it determines the token->expert dispatch, i.e. the sharding.
  - Tensor-parallel over the FFN intermediate dim across 8 cores: core c owns
    I-slice [c*512:(c+1)*512) of every expert (ws rows for gate and up, w2s cols).
  - Top-2 sparsity: tokens are packed per expert (padded to 256-token blocks);
    each core runs gate/up matmuls (contraction D=2048), SwiGLU, down matmul
    (contraction I_shard=512), scales rows by combine weights, and writes the
    packed rows contiguously.
  - Matmuls run in fp32r (11-bit mantissa, full PE rate at free dim >= 256).
    All weight/activation inputs are pre-rounded to fp32r on host (bit-exact
    with the device rounding); the on-chip h = silu(gate)*up write rounds to
    fp32r for free via the DVE output dtype.
  - A ReduceScatter over the 8 cores sums the I-shard partials of the packed
    rows; core c returns packed rows [c*npad/8:(c+1)*npad/8). The host
    concatenates the shards and assembles out[t] = packed[pos0[t]] +
    packed[pos1[t]] (the two expert contributions, already weighted on device).
"""

import math

import numpy as np

T = 4096
D = 2048
E = 8
I = 4096
TOPK = 2
NCORES = 8
ISH = I // NCORES  # 512, per-core I shard
BLK = 256  # token block (matmul free dim for gate/up)
P = 128
DCH = D // P  # 16 d-chunks
ICH = ISH // P  # 4 i-chunks


def _round_fp32r(x: np.ndarray) -> np.ndarray:
    """Round-to-nearest-even to 11 explicit mantissa bits (device-verified bit-exact)."""
    b = np.ascontiguousarray(x, dtype=np.float32).view(np.uint32).astype(np.uint64)
    bias = ((b >> 12) & 1) + np.uint64(0x7FF)
    r = ((b + bias) >> 12 << 12).astype(np.uint32)
    return r.view(np.float32)


def _host_router(x, router_w):
    """Replicate reference routing in numpy (fp32)."""
    logits = (x.astype(np.float64) @ router_w.astype(np.float64).T).astype(np.float32)
    m = logits.max(axis=-1, keepdims=True)
    ex = np.exp((logits - m).astype(np.float32))
    probs = ex / ex.sum(axis=-1, keepdims=True)
    # top-2, ties to lower index (matches jax.lax.top_k)
    top1 = probs.argmax(axis=-1)
    p = probs.copy()
    p[np.arange(T), top1] = -1.0
    top2 = p.argmax(axis=-1)
    w1 = probs[np.arange(T), top1]
    w2 = probs[np.arange(T), top2]
    s = w1 + w2
    return top1.astype(np.int64), top2.astype(np.int64), (w1 / s).astype(np.float32), (w2 / s).astype(np.float32)


_CACHE: dict = {}


def _build_bass(nblk: list[int], npad: int):
    """Build the 8-core SPMD Bass program. nblk[e] = number of 256-token blocks
    for expert e; npad = total packed (padded) tokens."""
    import concourse.bacc as bacc
    import concourse.mybir as mybir
    import concourse.tile as tile

    f32 = mybir.dt.float32
    f32r = mybir.dt.float32r
    nsub = npad // P  # 128-row subblocks

    nc = bacc.Bacc("TRN2", target_bir_lowering=False)

    nblk_tot = npad // BLK
    xtp_d = nc.dram_tensor("xtp", [P, nblk_tot, DCH, BLK], f32r, kind="ExternalInput")
    wst_d = nc.dram_tensor("wst", [E, DCH, P, 2 * ISH], f32r, kind="ExternalInput")
    w2st_d = nc.dram_tensor("w2st", [E, ICH, P, D], f32r, kind="ExternalInput")
    cw_d = nc.dram_tensor("cw", [P, nsub], f32, kind="ExternalInput")
    out_d = nc.dram_tensor("out", [npad // NCORES, D], f32, kind="ExternalOutput")

    with tile.TileContext(nc) as tc:
        with (
            tc.tile_pool(name="dram", bufs=1, space="DRAM") as dram_pool,
            tc.tile_pool(name="wpool", bufs=23) as wpool,
            tc.tile_pool(name="w2pool", bufs=5) as w2pool,
            tc.tile_pool(name="xpool", bufs=2) as xpool,
            tc.tile_pool(name="spool", bufs=3) as spool,
            tc.tile_pool(name="hpool", bufs=2) as hpool,
            tc.tile_pool(name="opool", bufs=2) as opool,
            tc.tile_pool(name="const", bufs=1) as const_pool,
            tc.tile_pool(name="ph", bufs=6, space="PSUM") as ph_pool,
            tc.tile_pool(name="po", bufs=2, space="PSUM") as po_pool,
        ):
            packed = dram_pool.tile([npad, D], f32)
            rs_out = dram_pool.tile([npad // NCORES, D], f32)

            # first token block issued before any weights so the first matmul's
            # deps (xt0 + wst tile 0) are at the head of the DMA queue
            xt0 = xpool.tile([P, DCH, BLK], f32r, tag="xt")
            nc.sync.dma_start(xt0[:], xtp_d[:, 0])

            # combine weights, resident (needed first at phase 3 of block 0)
            cw_sb = const_pool.tile([P, nsub], f32)
            nc.sync.dma_start(cw_sb[:], cw_d[:])

            gblk = 0
            for e in range(E):
                wst_tiles = []
                for dc in range(DCH):
                    wt = wpool.tile([P, 2 * ISH], f32r, tag="wst")
                    nc.sync.dma_start(wt[:], wst_d[e, dc])
                    wst_tiles.append(wt)
                w2_tiles = []
                for ic in range(ICH):
                    w2t = w2pool.tile([P, D], f32r, tag="w2st")
                    nc.sync.dma_start(w2t[:], w2st_d[e, ic])
                    w2_tiles.append(w2t)

                for _b in range(nblk[e]):
                    if gblk == 0:
                        xt = xt0
                    else:
                        xt = xpool.tile([P, DCH, BLK], f32r, tag="xt")
                        nc.sync.dma_start(xt[:], xtp_d[:, gblk])

                    # phase 1: gate/up in ic-pairs; each accumulation group gets
                    # its own PSUM bank (start=True clears the whole bank)
                    hT = hpool.tile([P, ICH, BLK], f32r, tag="hT")
                    for half in range(ICH // 2):
                        phg = [
                            ph_pool.tile([P, BLK], f32, tag="ph", name=f"phg_{gblk}_{half}_{j}")
                            for j in range(2)
                        ]
                        phu = [
                            ph_pool.tile([P, BLK], f32, tag="ph", name=f"phu_{gblk}_{half}_{j}")
                            for j in range(2)
                        ]
                        for dc in range(DCH):
                            wt = wst_tiles[dc]
                            for j in range(2):
                                ic = half * 2 + j
                                nc.tensor.matmul(
                                    phg[j][:],
                                    wt[:, ic * P : (ic + 1) * P],
                                    xt[:, dc, :],
                                    start=(dc == 0),
                                    stop=(dc == DCH - 1),
                                )
                                nc.tensor.matmul(
                                    phu[j][:],
                                    wt[:, ISH + ic * P : ISH + (ic + 1) * P],
                                    xt[:, dc, :],
                                    start=(dc == 0),
                                    stop=(dc == DCH - 1),
                                )
                        for j in range(2):
                            ic = half * 2 + j
                            sg = spool.tile([P, BLK], f32, tag="sg")
                            nc.scalar.activation(
                                sg[:], phg[j][:], mybir.ActivationFunctionType.Silu
                            )
                            nc.vector.tensor_mul(hT[:, ic, :], sg[:], phu[j][:])

                    # phase 3: down proj per 128-token subblock
                    for s in range(BLK // P):
                        gsub = gblk * (BLK // P) + s
                        osb = opool.tile([P, D], f32, tag="osb")
                        for dt_i in range(D // 512):
                            po_t = po_pool.tile([P, 512], f32, tag="po")
                            for ic in range(ICH):
                                nc.tensor.matmul(
                                    po_t[:],
                                    hT[:, ic, s * P : (s + 1) * P],
                                    w2_tiles[ic][:, dt_i * 512 : (dt_i + 1) * 512],
                                    start=(ic == 0),
                                    stop=(ic == ICH - 1),
                                )
                            # evacuate + scale by combine weight (split ACT/DVE)
                            if dt_i < 2:
                                nc.scalar.activation(
                                    osb[:, dt_i * 512 : (dt_i + 1) * 512],
                                    po_t[:],
                                    mybir.ActivationFunctionType.Copy,
                                    scale=cw_sb[:, gsub : gsub + 1],
                                )
                            else:
                                nc.vector.tensor_scalar_mul(
                                    osb[:, dt_i * 512 : (dt_i + 1) * 512],
                                    po_t[:],
                                    cw_sb[:, gsub : gsub + 1],
                                )
                        nc.sync.dma_start(
                            packed[gsub * P : (gsub + 1) * P, :], osb[:]
                        )
                    gblk += 1

                # expert e's packed rows are final on every core here; reduce-
                # scatter them now so the collective overlaps the next expert
                base = (gblk - nblk[e]) * BLK
                sz = nblk[e] * BLK
                nc.gpsimd.collective_compute(
                    "ReduceScatter",
                    mybir.AluOpType.add,
                    replica_groups=[list(range(NCORES))],
                    ins=[packed[base : base + sz].opt()],
                    outs=[rs_out[base // NCORES : (base + sz) // NCORES].opt()],
                )
            nc.sync.dma_start(out_d[:], rs_out[:])

    nc.compile()
    return nc


def _prepare(hidden_states, router_w, ws, w2s):
    """Host-side routing, packing, transposes, fp32r rounding. Returns
    (nblk, npad, pos, shared inputs dict, per-core weight arrays)."""
    x = np.asarray(hidden_states, dtype=np.float32).reshape(T, D)
    router_w = np.asarray(router_w, dtype=np.float32)
    ws = np.asarray(ws, dtype=np.float32)
    w2s = np.asarray(w2s, dtype=np.float32)

    top1, top2, w1, w2 = _host_router(x, router_w)

    # per-expert token lists and weights
    toks: list[list[int]] = [[] for _ in range(E)]
    cws: list[list[float]] = [[] for _ in range(E)]
    for ti, wi in [(top1, w1), (top2, w2)]:
        for t in range(T):
            e = int(ti[t])
            toks[e].append(t)
            cws[e].append(float(wi[t]))

    nblk = []
    perm = []
    cw = []
    # pos[k, t] = packed position of token t's k-th expert contribution
    pos = np.zeros((TOPK, T), dtype=np.int64)
    seen = np.zeros(T, dtype=np.int64)
    for e in range(E):
        n = len(toks[e])
        npd = math.ceil(n / BLK) * BLK if n > 0 else 0
        nblk.append(npd // BLK)
        base = len(perm)
        for j, t in enumerate(toks[e]):
            pos[seen[t], t] = base + j
            seen[t] += 1
        perm.extend(toks[e])
        cw.extend(cws[e])
        perm.extend([0] * (npd - n))
        cw.extend([0.0] * (npd - n))
    npad = len(perm)
    perm = np.asarray(perm, dtype=np.int64)

    # packed-transposed tokens, block-contiguous per partition:
    # xtp[p, b, dc, j] = x[perm[b*BLK + j], dc*128 + p]
    xr = _round_fp32r(x)
    nblk_tot = npad // BLK
    xtp = np.ascontiguousarray(
        xr[perm].reshape(nblk_tot, BLK, DCH, P).transpose(3, 0, 2, 1)
    )  # [P, nblk_tot, DCH, BLK]

    nsub = npad // P
    cw_a = np.asarray(cw, dtype=np.float32).reshape(nsub, P).T.copy()  # [P, nsub]

    # per-core weights
    wst_all = []
    w2st_all = []
    gate = ws[:, :I, :]  # [E, I, D]
    up = ws[:, I:, :]
    for c in range(NCORES):
        lo, hi = c * ISH, (c + 1) * ISH
        # [E, DCH, P, 2*ISH]: [.., d-part, gate(ISH)||up(ISH)]
        g = gate[:, lo:hi, :].reshape(E, ISH, DCH, P).transpose(0, 2, 3, 1)
        u = up[:, lo:hi, :].reshape(E, ISH, DCH, P).transpose(0, 2, 3, 1)
        wst = np.concatenate([g, u], axis=3)
        wst_all.append(_round_fp32r(np.ascontiguousarray(wst)))
        # w2s[e] is [D, I]; w2sT slice = w2s[:, :, lo:hi].T -> [E, ISH, D] -> [E, ICH, P, D]
        w2t = w2s[:, :, lo:hi].transpose(0, 2, 1).reshape(E, ICH, P, D)
        w2st_all.append(_round_fp32r(np.ascontiguousarray(w2t)))

    shared = {"xtp": xtp, "cw": cw_a}
    return nblk, npad, pos, shared, wst_all, w2st_all


def kernel(hidden_states, router_w, ws, w2s):
    from concourse import bass_utils

    hs = np.asarray(hidden_states)
    B, S, _ = hs.shape
    nblk, npad, pos, shared, wst_all, w2st_all = _prepare(hidden_states, router_w, ws, w2s)

    key = (tuple(nblk), npad)
    if key not in _CACHE:
        _CACHE[key] = _build_bass(nblk, npad)
    nc = _CACHE[key]

    in_maps = [
        {**shared, "wst": wst_all[c], "w2st": w2st_all[c]} for c in range(NCORES)
    ]
    res = bass_utils.run_bass_kernel_spmd(nc, in_maps, core_ids=list(range(NCORES)))
    # per-expert chunked RS: within each expert's row range, core c holds the
    # c-th eighth; reassemble the full packed array
    npad_total = sum(nblk) * BLK
    packed = np.empty((npad_total, D), dtype=np.float32)
    base = 0
    for e in range(E):
        sz = nblk[e] * BLK
        sz8 = sz // NCORES
        for c in range(NCORES):
            packed[base + c * sz8 : base + (c + 1) * sz8] = res.results[c]["out"][
                base // NCORES : base // NCORES + sz8
            ]
        base += sz
    out = packed[pos[0]] + packed[pos[1]]  # the two (device-weighted) expert contributions
    return out.reshape(B, S, D).astype(np.float32)



# revision 2
# speedup vs baseline: 1.3111x; 1.3111x over previous
"""DBRX MoE experts kernel for Trainium2 (8 NeuronCores).

Strategy (v2):
  - Router (logits -> softmax -> top-2 -> renormalize) computed on host in numpy
    (0.01% of FLOPs); it determines the token->expert dispatch, i.e. the sharding.
  - Tensor-parallel over the FFN intermediate dim across 8 cores: core c owns
    I-slice [c*512:(c+1)*512) of every expert (ws rows for gate and up, w2s cols).
  - Top-2 sparsity: tokens are packed per expert EXACTLY (no padding): per
    expert, chunks of 256 tokens plus one ragged tail chunk (w = n_e mod 256).
    Each core runs gate/up matmuls (contraction D=2048), SwiGLU, down matmul
    (contraction I_shard=512), scales rows by combine weights, and writes the
    packed rows to its own full-size partial output.
  - All matmul operands are fp16 (1.0 cycles/row on the PE at any free size,
    half the HBM traffic of fp32); accumulation is fp32 in PSUM.
  - No on-device collective: each core writes its partial [8192, D] output
    (fp16) and the host sums the 8 partials and gathers the two expert
    contributions per token.
  - Pipelining: per chunk c the program issues P1(c) (gate/up+SwiGLU) then
    Down(c-1), so the PE never waits on the DVE/ACT producing h. Input loads
    (x chunks, weights) ride the SP sequencer queue with weights prefetched
    one expert ahead; output stores ride the otherwise-idle gpsimd (SWDGE)
    queue so they never stall input loads.
"""

import math

import numpy as np

T = 4096
D = 2048
E = 8
I = 4096
TOPK = 2
NCORES = 8
ISH = I // NCORES  # 512, per-core I shard
P = 128
DCH = D // P  # 16 d-chunks
ICH = ISH // P  # 4 i-chunks
CHUNK = 256  # max token chunk (matmul free dim for gate/up)


def _host_router(x, router_w):
    """Replicate reference routing in numpy (fp32)."""
    logits = (x.astype(np.float64) @ router_w.astype(np.float64).T).astype(np.float32)
    m = logits.max(axis=-1, keepdims=True)
    ex = np.exp((logits - m).astype(np.float32))
    probs = ex / ex.sum(axis=-1, keepdims=True)
    # top-2, ties to lower index (matches jax.lax.top_k)
    top1 = probs.argmax(axis=-1)
    p = probs.copy()
    p[np.arange(T), top1] = -1.0
    top2 = p.argmax(axis=-1)
    w1 = probs[np.arange(T), top1]
    w2 = probs[np.arange(T), top2]
    s = w1 + w2
    return top1.astype(np.int64), top2.astype(np.int64), (w1 / s).astype(np.float32), (w2 / s).astype(np.float32)


_CACHE: dict = {}


def _build_bass(chunks):
    """Build the 8-core SPMD Bass program.

    chunks: tuple of (expert, width) in packed-token order; widths sum to T*TOPK.
    """
    import concourse.bacc as bacc
    import concourse.mybir as mybir
    import concourse.tile as tile

    f32 = mybir.dt.float32
    f16 = mybir.dt.float16

    nchunks = len(chunks)
    xlen = sum(DCH * w for _, w in chunks)
    ncols = sum((w + P - 1) // P for _, w in chunks)
    ntok = sum(w for _, w in chunks)

    # per-chunk packed-token base, x offset, cw column base
    tokbase = []
    xoff = []
    colbase = []
    tb = xo = cb = 0
    for _, w in chunks:
        tokbase.append(tb)
        xoff.append(xo)
        colbase.append(cb)
        tb += w
        xo += DCH * w
        cb += (w + P - 1) // P

    # expert schedule: unique experts in chunk order, with local chunk counts
    experts_used = []
    for e, _ in chunks:
        if not experts_used or experts_used[-1] != e:
            experts_used.append(e)
    nch_of = {e: sum(1 for ee, _ in chunks if ee == e) for e in experts_used}
    ei_of_chunk = []  # expert-INDEX per global chunk
    loc_of_chunk = []  # local chunk index within its expert
    cur = -1
    loc = 0
    for e, _ in chunks:
        if cur == -1 or experts_used[cur] != e:
            cur += 1
            loc = 0
        ei_of_chunk.append(cur)
        loc_of_chunk.append(loc)
        loc += 1

    nc = bacc.Bacc("TRN2", target_bir_lowering=False)

    xtp_d = nc.dram_tensor("xtp", [P, xlen], f16, kind="ExternalInput")
    wst_d = nc.dram_tensor("wst", [E, DCH, P, 2 * ISH], f16, kind="ExternalInput")
    w2st_d = nc.dram_tensor("w2st", [E, ICH, P, D], f16, kind="ExternalInput")
    cw_d = nc.dram_tensor("cw", [P, ncols], f32, kind="ExternalInput")
    out_d = nc.dram_tensor("out", [ntok, D], f16, kind="ExternalOutput")

    with tile.TileContext(nc) as tc:
        with (
            tc.tile_pool(name="wpool", bufs=32) as wpool,
            tc.tile_pool(name="w2pool", bufs=12) as w2pool,
            tc.tile_pool(name="xpool", bufs=3) as xpool,
            tc.tile_pool(name="spool", bufs=3) as spool,
            tc.tile_pool(name="hpool", bufs=2) as hpool,
            tc.tile_pool(name="opool", bufs=3) as opool,
            tc.tile_pool(name="const", bufs=1) as const_pool,
            tc.tile_pool(name="ph", bufs=4, space="PSUM") as ph_pool,
            tc.tile_pool(name="po", bufs=3, space="PSUM") as po_pool,
        ):
            wtiles: dict = {}
            w2tiles: dict = {}

            def wst_thunk(e, dc):
                def run():
                    t = wpool.tile([P, 2 * ISH], f16, tag="wst", name=f"wst{e}_{dc}")
                    nc.sync.dma_start(t[:], wst_d[e, dc])
                    wtiles.setdefault(e, {})[dc] = t

                return run

            def w2_thunk(e, ic):
                def run():
                    t = w2pool.tile([P, D], f16, tag="w2st", name=f"w2st{e}_{ic}")
                    nc.sync.dma_start(t[:], w2st_d[e, ic])
                    w2tiles.setdefault(e, {})[ic] = t

                return run

            # per expert-index: list of 20 weight-DMA thunks (wst dc0..15, w2 ic0..3)
            wthunks = [
                [wst_thunk(e, dc) for dc in range(DCH)]
                + [w2_thunk(e, ic) for ic in range(ICH)]
                for e in experts_used
            ]
            wissued = [0] * len(experts_used)

            def issue_weights(i, upto):
                while wissued[i] < upto:
                    wthunks[i][wissued[i]]()
                    wissued[i] += 1

            xts: dict = {}

            def issue_xt(g):
                e, w = chunks[g]
                t = xpool.tile([P, DCH * CHUNK], f16, tag="xt", name=f"xt{g}")
                nc.sync.dma_start(t[:, : DCH * w], xtp_d[:, xoff[g] : xoff[g] + DCH * w])
                xts[g] = t

            # startup: first x chunk, expert-0 gate/up weights, second x chunk,
            # combine weights, expert-0 down weights
            issue_xt(0)
            issue_weights(0, DCH)
            if nchunks > 1:
                issue_xt(1)
            cw_sb = const_pool.tile([P, ncols], f32)
            nc.sync.dma_start(cw_sb[:], cw_d[:])
            issue_weights(0, DCH + ICH)

            hTs: dict = {}

            def phase1(g):
                e, w = chunks[g]
                xt = xts.pop(g)
                hT = hpool.tile([P, ICH * CHUNK], f16, tag="hT", name=f"hT{g}")
                for ic in range(ICH):
                    pg = ph_pool.tile([P, CHUNK], f32, tag="ph", name=f"pg{g}_{ic}")
                    pu = ph_pool.tile([P, CHUNK], f32, tag="ph", name=f"pu{g}_{ic}")
                    for dc in range(DCH):
                        wt = wtiles[e][dc]
                        xs = xt[:, dc * w : (dc + 1) * w]
                        nc.tensor.matmul(
                            pg[:, :w],
                            wt[:, ic * P : (ic + 1) * P],
                            xs,
                            start=(dc == 0),
                            stop=(dc == DCH - 1),
                        )
                        nc.tensor.matmul(
                            pu[:, :w],
                            wt[:, ISH + ic * P : ISH + (ic + 1) * P],
                            xs,
                            start=(dc == 0),
                            stop=(dc == DCH - 1),
                        )
                    sg = spool.tile([P, CHUNK], f16, tag="sg", name=f"sg{g}_{ic}")
                    nc.scalar.activation(
                        sg[:, :w], pg[:, :w], mybir.ActivationFunctionType.Silu
                    )
                    nc.vector.tensor_mul(
                        hT[:, ic * CHUNK : ic * CHUNK + w], sg[:, :w], pu[:, :w]
                    )
                hTs[g] = hT

            def down(g):
                e, w = chunks[g]
                hT = hTs.pop(g)
                nsb = (w + P - 1) // P
                for sb in range(nsb):
                    s = min(P, w - sb * P)
                    osb = opool.tile([P, D], f16, tag="osb", name=f"osb{g}_{sb}")
                    col = colbase[g] + sb
                    cw_ap = cw_sb[:s, col : col + 1]
                    for dt in range(D // 512):
                        po = po_pool.tile([P, 512], f32, tag="po", name=f"po{g}_{sb}_{dt}")
                        for ic in range(ICH):
                            nc.tensor.matmul(
                                po[:s, :],
                                hT[:, ic * CHUNK + sb * P : ic * CHUNK + sb * P + s],
                                w2tiles[e][ic][:, dt * 512 : (dt + 1) * 512],
                                start=(ic == 0),
                                stop=(ic == ICH - 1),
                            )
                        # evacuate + scale by combine weight (split ACT/DVE)
                        if dt < 2:
                            nc.scalar.activation(
                                osb[:s, dt * 512 : (dt + 1) * 512],
                                po[:s, :],
                                mybir.ActivationFunctionType.Copy,
                                scale=cw_ap,
                            )
                        else:
                            nc.vector.tensor_scalar_mul(
                                osb[:s, dt * 512 : (dt + 1) * 512], po[:s, :], cw_ap
                            )
                    # output store on the gpsimd (SWDGE) queue: never blocks
                    # the SP input-load queue
                    base = tokbase[g] + sb * P
                    nc.gpsimd.dma_start(out_d[base : base + s, :], osb[:s, :])

            for g in range(nchunks):
                if g + 2 < nchunks:
                    issue_xt(g + 2)
                # prefetch next expert's weights, paced across this expert's chunks
                i = ei_of_chunk[g]
                if i + 1 < len(experts_used):
                    m = nch_of[experts_used[i]]
                    j = loc_of_chunk[g]
                    issue_weights(i + 1, math.ceil(20 * (j + 1) / m))
                phase1(g)
                if g > 0:
                    down(g - 1)
            down(nchunks - 1)

    nc.compile()
    return nc


def _prepare(hidden_states, router_w, ws, w2s):
    """Host-side routing, packing, transposes, fp16 casts. Returns
    (chunks, pos, shared inputs dict, per-core weight arrays)."""
    x = np.asarray(hidden_states, dtype=np.float32).reshape(T, D)
    router_w = np.asarray(router_w, dtype=np.float32)
    ws = np.asarray(ws, dtype=np.float32)
    w2s = np.asarray(w2s, dtype=np.float32)

    top1, top2, w1, w2 = _host_router(x, router_w)

    # per-expert token lists and weights
    toks: list[list[int]] = [[] for _ in range(E)]
    cws: list[list[float]] = [[] for _ in range(E)]
    for ti, wi in [(top1, w1), (top2, w2)]:
        for t in range(T):
            e = int(ti[t])
            toks[e].append(t)
            cws[e].append(float(wi[t]))

    # exact packing: per expert, 256-token chunks + ragged tail
    chunks: list[tuple[int, int]] = []
    perm: list[int] = []
    cwf: list[float] = []
    pos = np.zeros((TOPK, T), dtype=np.int64)
    seen = np.zeros(T, dtype=np.int64)
    for e in range(E):
        n = len(toks[e])
        if n == 0:
            continue
        base = len(perm)
        for j, t in enumerate(toks[e]):
            pos[seen[t], t] = base + j
            seen[t] += 1
        perm.extend(toks[e])
        cwf.extend(cws[e])
        nfull, tail = divmod(n, CHUNK)
        chunks.extend([(e, CHUNK)] * nfull)
        if tail:
            chunks.append((e, tail))
    ntok = len(perm)
    perm_a = np.asarray(perm, dtype=np.int64)

    # packed-transposed tokens, fp16, chunk-contiguous per partition:
    # per chunk (w tokens): xtp[p, off + dc*w + j] = x[perm[tb + j], dc*128 + p]
    xb = x[perm_a].astype(np.float16)  # [ntok, D]
    xlen = DCH * ntok
    xtp = np.empty((P, xlen), dtype=np.float16)
    tb = xo = 0
    for _, w in chunks:
        blk = xb[tb : tb + w].reshape(w, DCH, P).transpose(2, 1, 0).reshape(P, DCH * w)
        xtp[:, xo : xo + DCH * w] = blk
        tb += w
        xo += DCH * w

    # combine weights: one [P] column per (chunk, 128-subblock)
    ncols = sum((w + P - 1) // P for _, w in chunks)
    cw_a = np.zeros((P, ncols), dtype=np.float32)
    cw_flat = np.asarray(cwf, dtype=np.float32)
    tb = col = 0
    for _, w in chunks:
        for sb in range((w + P - 1) // P):
            s = min(P, w - sb * P)
            cw_a[:s, col] = cw_flat[tb + sb * P : tb + sb * P + s]
            col += 1
        tb += w

    # per-core weights (fp16)
    wst_all = []
    w2st_all = []
    gate = ws[:, :I, :]  # [E, I, D]
    up = ws[:, I:, :]
    for c in range(NCORES):
        lo, hi = c * ISH, (c + 1) * ISH
        # [E, DCH, P, 2*ISH]: [.., d-part, gate(ISH)||up(ISH)]
        g = gate[:, lo:hi, :].reshape(E, ISH, DCH, P).transpose(0, 2, 3, 1)
        u = up[:, lo:hi, :].reshape(E, ISH, DCH, P).transpose(0, 2, 3, 1)
        wst = np.concatenate([g, u], axis=3)
        wst_all.append(np.ascontiguousarray(wst, dtype=np.float16))
        # w2s[e] is [D, I]; rhs tile [ic, p(i), d] = w2s[e, d, lo + ic*128 + p]
        w2t = w2s[:, :, lo:hi].transpose(0, 2, 1).reshape(E, ICH, P, D)
        w2st_all.append(np.ascontiguousarray(w2t, dtype=np.float16))

    shared = {"xtp": xtp, "cw": cw_a}
    return tuple(chunks), ntok, pos, shared, wst_all, w2st_all


def kernel(hidden_states, router_w, ws, w2s):
    from concourse import bass_utils

    hs = np.asarray(hidden_states)
    B, S, _ = hs.shape
    chunks, ntok, pos, shared, wst_all, w2st_all = _prepare(
        hidden_states, router_w, ws, w2s
    )

    if chunks not in _CACHE:
        _CACHE[chunks] = _build_bass(chunks)
    nc = _CACHE[chunks]

    in_maps = [
        {**shared, "wst": wst_all[c], "w2st": w2st_all[c]} for c in range(NCORES)
    ]
    res = bass_utils.run_bass_kernel_spmd(nc, in_maps, core_ids=list(range(NCORES)))
    # host combine: sum the 8 I-shard partials, then gather the two
    # (device-weighted) expert contributions per token
    packed = np.zeros((ntok, D), dtype=np.float32)
    for c in range(NCORES):
        packed += res.results[c]["out"].astype(np.float32)
    out = packed[pos[0]] + packed[pos[1]]
    return out.reshape(B, S, D).astype(np.float32)


# revision 3
# speedup vs baseline: 1.3217x; 1.0081x over previous
"""DBRX MoE experts kernel for Trainium2 (8 NeuronCores).

Strategy (v3):
  - Router (logits -> softmax -> top-2 -> renormalize) computed on host in numpy
    (0.01% of FLOPs); it determines the token->expert dispatch, i.e. the sharding.
  - Tensor-parallel over the FFN intermediate dim across 8 cores: core c owns
    I-slice [c*512:(c+1)*512) of every expert (ws rows for gate and up, w2s cols).
  - Top-2 sparsity: tokens are packed per expert EXACTLY (no padding): per
    expert, chunks of <=256 tokens (full 256-chunks plus one ragged tail).
  - All matmul operands are fp16 (1.0 PE cycles/row at any free size, half the
    HBM traffic of fp32); accumulation is fp32 in PSUM. Every matmul streams
    N=w tokens (gate/up: [128i x w], down: [128d x w]), so PE time is exactly
    proportional to routed tokens: no padding waste anywhere.
  - The combine weights are applied on the HOST during the final gather
    (out[t] = cw0[t]*packed[pos0[t]] + cw1[t]*packed[pos1[t]]), so the device
    writes unscaled partials and PSUM evacuation is a plain copy.
  - No on-device collective: each core writes its partial output in a
    d-major transposed layout [128, 16, ntok] (fp16); the host sums the 8
    partials, transposes, scales and gathers.
  - Pipelining: per chunk c the program issues P1(c) (gate/up+SwiGLU) then
    Down(c-1), so the PE never waits on the DVE/ACT producing h. Input loads
    (x chunks, weights) ride the SP sequencer queue with weights prefetched
    one expert ahead; output stores ride the ACT queue (issued after that
    chunk's evacuation copies), so they never stall input loads.
"""

import math

import numpy as np

T = 4096
D = 2048
E = 8
I = 4096
TOPK = 2
NCORES = 8
ISH = I // NCORES  # 512, per-core I shard
P = 128
DCH = D // P  # 16 d-chunks
ICH = ISH // P  # 4 i-chunks
CHUNK = 256  # max token chunk (matmul moving free dim)


def _host_router(x, router_w):
    """Replicate reference routing in numpy (fp32)."""
    logits = (x.astype(np.float64) @ router_w.astype(np.float64).T).astype(np.float32)
    m = logits.max(axis=-1, keepdims=True)
    ex = np.exp((logits - m).astype(np.float32))
    probs = ex / ex.sum(axis=-1, keepdims=True)
    # top-2, ties to lower index (matches jax.lax.top_k)
    top1 = probs.argmax(axis=-1)
    p = probs.copy()
    p[np.arange(T), top1] = -1.0
    top2 = p.argmax(axis=-1)
    w1 = probs[np.arange(T), top1]
    w2 = probs[np.arange(T), top2]
    s = w1 + w2
    return top1.astype(np.int64), top2.astype(np.int64), (w1 / s).astype(np.float32), (w2 / s).astype(np.float32)


_CACHE: dict = {}


def _build_bass(chunks):
    """Build the 8-core SPMD Bass program.

    chunks: tuple of (expert, width) in packed-token order; widths sum to T*TOPK.
    """
    import concourse.bacc as bacc
    import concourse.mybir as mybir
    import concourse.tile as tile

    f16 = mybir.dt.float16

    nchunks = len(chunks)
    xlen = sum(DCH * w for _, w in chunks)
    ntok = sum(w for _, w in chunks)

    # per-chunk packed-token base and x offset
    tokbase = []
    xoff = []
    tb = xo = 0
    for _, w in chunks:
        tokbase.append(tb)
        xoff.append(xo)
        tb += w
        xo += DCH * w

    # expert schedule: unique experts in chunk order, with local chunk counts
    experts_used = []
    for e, _ in chunks:
        if not experts_used or experts_used[-1] != e:
            experts_used.append(e)
    nch_of = {e: sum(1 for ee, _ in chunks if ee == e) for e in experts_used}
    ei_of_chunk = []  # expert-INDEX per global chunk
    loc_of_chunk = []  # local chunk index within its expert
    cur = -1
    loc = 0
    for e, _ in chunks:
        if cur == -1 or experts_used[cur] != e:
            cur += 1
            loc = 0
        ei_of_chunk.append(cur)
        loc_of_chunk.append(loc)
        loc += 1

    nc = bacc.Bacc("TRN2", target_bir_lowering=False)

    xtp_d = nc.dram_tensor("xtp", [P, xlen], f16, kind="ExternalInput")
    wst_d = nc.dram_tensor("wst", [E, DCH, P, 2 * ISH], f16, kind="ExternalInput")
    w2st_d = nc.dram_tensor("w2st", [E, ICH, P, D], f16, kind="ExternalInput")
    # partial output, d-major transposed: out[p, c, t] = partial[t, c*128 + p]
    out_d = nc.dram_tensor("out", [P, DCH, ntok], f16, kind="ExternalOutput")

    with tile.TileContext(nc) as tc:
        with (
            tc.tile_pool(name="wpool", bufs=32) as wpool,
            tc.tile_pool(name="w2pool", bufs=12) as w2pool,
            tc.tile_pool(name="xpool", bufs=3) as xpool,
            tc.tile_pool(name="spool", bufs=3) as spool,
            tc.tile_pool(name="hpool", bufs=2) as hpool,
            tc.tile_pool(name="opool", bufs=3) as opool,
            tc.tile_pool(name="ph", bufs=4, space="PSUM") as ph_pool,
            tc.tile_pool(name="po", bufs=3, space="PSUM") as po_pool,
        ):
            wtiles: dict = {}
            w2tiles: dict = {}

            def wst_thunk(e, dc):
                def run():
                    t = wpool.tile([P, 2 * ISH], f16, tag="wst", name=f"wst{e}_{dc}")
                    nc.sync.dma_start(t[:], wst_d[e, dc])
                    wtiles.setdefault(e, {})[dc] = t

                return run

            def w2_thunk(e, ic):
                def run():
                    t = w2pool.tile([P, D], f16, tag="w2st", name=f"w2st{e}_{ic}")
                    nc.sync.dma_start(t[:], w2st_d[e, ic])
                    w2tiles.setdefault(e, {})[ic] = t

                return run

            # per expert-index: list of 20 weight-DMA thunks (wst dc0..15, w2 ic0..3)
            wthunks = [
                [wst_thunk(e, dc) for dc in range(DCH)]
                + [w2_thunk(e, ic) for ic in range(ICH)]
                for e in experts_used
            ]
            wissued = [0] * len(experts_used)

            def issue_weights(i, upto):
                upto = min(upto, len(wthunks[i]))
                while wissued[i] < upto:
                    wthunks[i][wissued[i]]()
                    wissued[i] += 1

            xts: dict = {}

            def issue_xt(g):
                e, w = chunks[g]
                t = xpool.tile([P, DCH * CHUNK], f16, tag="xt", name=f"xt{g}")
                nc.sync.dma_start(t[:, : DCH * w], xtp_d[:, xoff[g] : xoff[g] + DCH * w])
                xts[g] = t

            # startup: first x chunk, expert-0 gate/up weights, second x chunk,
            # expert-0 down weights
            issue_xt(0)
            issue_weights(0, DCH)
            if nchunks > 1:
                issue_xt(1)
            issue_weights(0, DCH + ICH)

            hTs: dict = {}

            def phase1(g):
                e, w = chunks[g]
                xt = xts.pop(g)
                hT = hpool.tile([P, ICH * CHUNK], f16, tag="hT", name=f"hT{g}")
                for ic in range(ICH):
                    pg = ph_pool.tile([P, CHUNK], mybir.dt.float32, tag="ph", name=f"pg{g}_{ic}")
                    pu = ph_pool.tile([P, CHUNK], mybir.dt.float32, tag="ph", name=f"pu{g}_{ic}")
                    for dc in range(DCH):
                        wt = wtiles[e][dc]
                        xs = xt[:, dc * w : (dc + 1) * w]
                        nc.tensor.matmul(
                            pg[:, :w],
                            wt[:, ic * P : (ic + 1) * P],
                            xs,
                            start=(dc == 0),
                            stop=(dc == DCH - 1),
                        )
                        nc.tensor.matmul(
                            pu[:, :w],
                            wt[:, ISH + ic * P : ISH + (ic + 1) * P],
                            xs,
                            start=(dc == 0),
                            stop=(dc == DCH - 1),
                        )
                    sg = spool.tile([P, CHUNK], f16, tag="sg", name=f"sg{g}_{ic}")
                    nc.scalar.activation(
                        sg[:, :w], pg[:, :w], mybir.ActivationFunctionType.Silu
                    )
                    nc.vector.tensor_mul(
                        hT[:, ic * CHUNK : ic * CHUNK + w], sg[:, :w], pu[:, :w]
                    )
                hTs[g] = hT

            def down(g):
                e, w = chunks[g]
                hT = hTs.pop(g)
                osb = opool.tile([P, DCH, CHUNK], f16, tag="osb", name=f"osb{g}")
                for dc in range(DCH):
                    po = po_pool.tile([P, CHUNK], mybir.dt.float32, tag="po", name=f"po{g}_{dc}")
                    for ic in range(ICH):
                        nc.tensor.matmul(
                            po[:, :w],
                            w2tiles[e][ic][:, dc * P : (dc + 1) * P],
                            hT[:, ic * CHUNK : ic * CHUNK + w],
                            start=(ic == 0),
                            stop=(ic == ICH - 1),
                        )
                    # evacuate PSUM -> SBUF fp16 (plain copy; combine weights
                    # are applied on the host). Split ACT/DVE.
                    if dc % 2 == 0:
                        nc.scalar.activation(
                            osb[:, dc, :w], po[:, :w], mybir.ActivationFunctionType.Copy
                        )
                    else:
                        nc.vector.tensor_copy(osb[:, dc, :w], po[:, :w])
                # output store on the ACT queue (after its own evac copies):
                # never blocks the SP input-load queue
                base = tokbase[g]
                nc.scalar.dma_start(
                    out_d[:, :, base : base + w], osb[:, :, :w]
                )

            for g in range(nchunks):
                if g + 2 < nchunks:
                    issue_xt(g + 2)
                # prefetch next expert's weights, paced across this expert's chunks
                i = ei_of_chunk[g]
                if i + 1 < len(experts_used):
                    m = nch_of[experts_used[i]]
                    j = loc_of_chunk[g]
                    issue_weights(i + 1, math.ceil(20 * (j + 1) / m))
                phase1(g)
                if g > 0:
                    down(g - 1)
            down(nchunks - 1)

    nc.compile()
    return nc


def _prepare(hidden_states, router_w, ws, w2s):
    """Host-side routing, packing, transposes, fp16 casts. Returns
    (chunks, ntok, pos, topw, shared inputs dict, per-core weight arrays)."""
    x = np.asarray(hidden_states, dtype=np.float32).reshape(T, D)
    router_w = np.asarray(router_w, dtype=np.float32)
    ws = np.asarray(ws, dtype=np.float32)
    w2s = np.asarray(w2s, dtype=np.float32)

    top1, top2, w1, w2 = _host_router(x, router_w)

    # per-expert token lists
    toks: list[list[int]] = [[] for _ in range(E)]
    for ti in (top1, top2):
        for t in range(T):
            toks[int(ti[t])].append(t)

    # expert processing order: put the expert with the smallest nonzero tail
    # last, so the final chunk (and hence the program tail) is small
    tails = {e: (len(toks[e]) % CHUNK) for e in range(E) if len(toks[e]) > 0}
    order = [e for e in range(E) if len(toks[e]) > 0]
    if tails:
        last = min(tails, key=lambda e: (tails[e] == 0, tails[e]))
        order = [e for e in order if e != last] + [last]

    # exact packing: per expert, 256-token chunks + ragged tail
    chunks: list[tuple[int, int]] = []
    perm: list[int] = []
    pos = np.zeros((TOPK, T), dtype=np.int64)
    seen: dict[int, int] = {}
    for e in order:
        n = len(toks[e])
        base = len(perm)
        for j, t in enumerate(toks[e]):
            k = seen.get(t, 0)
            pos[k, t] = base + j
            seen[t] = k + 1
        perm.extend(toks[e])
        nfull, tail = divmod(n, CHUNK)
        chunks.extend([(e, CHUNK)] * nfull)
        if tail:
            chunks.append((e, tail))
    ntok = len(perm)
    perm_a = np.asarray(perm, dtype=np.int64)

    # NOTE: pos[k, t] maps (token, k-th choice) -> packed row, but the k-th
    # append pass order is (top1, top2); seen[] ensures pos rows follow that.
    topw = np.stack([w1, w2], axis=0)  # [2, T] renormalized weights

    # packed-transposed tokens, fp16, chunk-contiguous per partition:
    # per chunk (w tokens): xtp[p, off + dc*w + j] = x[perm[tb + j], dc*128 + p]
    xb = x[perm_a].astype(np.float16)  # [ntok, D]
    xlen = DCH * ntok
    xtp = np.empty((P, xlen), dtype=np.float16)
    tb = xo = 0
    for _, w in chunks:
        blk = xb[tb : tb + w].reshape(w, DCH, P).transpose(2, 1, 0).reshape(P, DCH * w)
        xtp[:, xo : xo + DCH * w] = blk
        tb += w
        xo += DCH * w

    # per-core weights (fp16)
    wst_all = []
    w2st_all = []
    gate = ws[:, :I, :]  # [E, I, D]
    up = ws[:, I:, :]
    for c in range(NCORES):
        lo, hi = c * ISH, (c + 1) * ISH
        # [E, DCH, P, 2*ISH]: [.., d-part, gate(ISH)||up(ISH)]
        g = gate[:, lo:hi, :].reshape(E, ISH, DCH, P).transpose(0, 2, 3, 1)
        u = up[:, lo:hi, :].reshape(E, ISH, DCH, P).transpose(0, 2, 3, 1)
        wst = np.concatenate([g, u], axis=3)
        wst_all.append(np.ascontiguousarray(wst, dtype=np.float16))
        # w2s[e] is [D, I]; lhsT tile [ic, p(i), d] = w2s[e, d, lo + ic*128 + p]
        w2t = w2s[:, :, lo:hi].transpose(0, 2, 1).reshape(E, ICH, P, D)
        w2st_all.append(np.ascontiguousarray(w2t, dtype=np.float16))

    shared = {"xtp": xtp}
    return tuple(chunks), ntok, pos, topw, shared, wst_all, w2st_all


def kernel(hidden_states, router_w, ws, w2s):
    from concourse import bass_utils

    hs = np.asarray(hidden_states)
    B, S, _ = hs.shape
    chunks, ntok, pos, topw, shared, wst_all, w2st_all = _prepare(
        hidden_states, router_w, ws, w2s
    )

    if chunks not in _CACHE:
        _CACHE[chunks] = _build_bass(chunks)
    nc = _CACHE[chunks]

    in_maps = [
        {**shared, "wst": wst_all[c], "w2st": w2st_all[c]} for c in range(NCORES)
    ]
    res = bass_utils.run_bass_kernel_spmd(nc, in_maps, core_ids=list(range(NCORES)))
    # host combine: sum the 8 I-shard partials (d-major transposed layout),
    # then scale by the combine weights and gather the two expert
    # contributions per token
    acc = np.zeros((P, DCH, ntok), dtype=np.float32)
    for c in range(NCORES):
        acc += res.results[c]["out"].astype(np.float32)
    packed = acc.transpose(2, 1, 0).reshape(ntok, D)  # [t, dc*128+p]
    out = topw[0][:, None] * packed[pos[0]] + topw[1][:, None] * packed[pos[1]]
    return out.reshape(B, S, D).astype(np.float32)


# revision 11
# speedup vs baseline: 1.3432x; 1.0163x over previous
"""DBRX MoE experts kernel for Trainium2 (8 NeuronCores).

Strategy (v3):
  - Router (logits -> softmax -> top-2 -> renormalize) computed on host in numpy
    (0.01% of FLOPs); it determines the token->expert dispatch, i.e. the sharding.
  - Tensor-parallel over the FFN intermediate dim across 8 cores: core c owns
    I-slice [c*512:(c+1)*512) of every expert (ws rows for gate and up, w2s cols).
  - Top-2 sparsity: tokens are packed per expert EXACTLY (no padding): per
    expert, chunks of <=256 tokens (full 256-chunks plus one ragged tail).
  - All matmul operands are fp16 (1.0 PE cycles/row at any free size, half the
    HBM traffic of fp32); accumulation is fp32 in PSUM. Every matmul streams
    N=w tokens (gate/up: [128i x w], down: [128d x w]), so PE time is exactly
    proportional to routed tokens: no padding waste anywhere.
  - The combine weights are applied on the HOST during the final gather
    (out[t] = cw0[t]*packed[pos0[t]] + cw1[t]*packed[pos1[t]]), so the device
    writes unscaled partials and PSUM evacuation is a plain copy.
  - No on-device collective: each core writes its partial output in a
    d-major transposed layout [128, 16, ntok] (fp16); the host sums the 8
    partials, transposes, scales and gathers.
  - Pipelining: per chunk c the program issues P1(c) (gate/up+SwiGLU) then
    Down(c-1), so the PE never waits on the DVE/ACT producing h. Input loads
    (x chunks, weights) ride the SP sequencer queue with weights prefetched
    one expert ahead; output stores ride the ACT queue (issued after that
    chunk's evacuation copies), so they never stall input loads.
"""

import math

import numpy as np

T = 4096
D = 2048
E = 8
I = 4096
TOPK = 2
NCORES = 8
ISH = I // NCORES  # 512, per-core I shard
P = 128
DCH = D // P  # 16 d-chunks
ICH = ISH // P  # 4 i-chunks
CHUNK = 512  # max token chunk (PSUM bank holds 512 fp32 per partition)


def _host_router(x, router_w):
    """Replicate reference routing in numpy (fp32)."""
    logits = (x.astype(np.float64) @ router_w.astype(np.float64).T).astype(np.float32)
    m = logits.max(axis=-1, keepdims=True)
    ex = np.exp((logits - m).astype(np.float32))
    probs = ex / ex.sum(axis=-1, keepdims=True)
    # top-2, ties to lower index (matches jax.lax.top_k)
    top1 = probs.argmax(axis=-1)
    p = probs.copy()
    p[np.arange(T), top1] = -1.0
    top2 = p.argmax(axis=-1)
    w1 = probs[np.arange(T), top1]
    w2 = probs[np.arange(T), top2]
    s = w1 + w2
    return top1.astype(np.int64), top2.astype(np.int64), (w1 / s).astype(np.float32), (w2 / s).astype(np.float32)


_CACHE: dict = {}


def _build_bass(chunks):
    """Build the 8-core SPMD Bass program.

    chunks: tuple of (expert, width) in packed-token order; widths sum to T*TOPK.
    """
    import concourse.bacc as bacc
    import concourse.mybir as mybir
    import concourse.tile as tile

    f16 = mybir.dt.float16

    nchunks = len(chunks)
    xlen = sum(DCH * w for _, w in chunks)
    ntok = sum(w for _, w in chunks)

    # per-chunk packed-token base and x offset
    tokbase = []
    xoff = []
    tb = xo = 0
    for _, w in chunks:
        tokbase.append(tb)
        xoff.append(xo)
        tb += w
        xo += DCH * w

    # expert schedule: unique experts in chunk order, with local chunk counts
    experts_used = []
    for e, _ in chunks:
        if not experts_used or experts_used[-1] != e:
            experts_used.append(e)
    nch_of = {e: sum(1 for ee, _ in chunks if ee == e) for e in experts_used}
    ei_of_chunk = []  # expert-INDEX per global chunk
    loc_of_chunk = []  # local chunk index within its expert
    cur = -1
    loc = 0
    for e, _ in chunks:
        if cur == -1 or experts_used[cur] != e:
            cur += 1
            loc = 0
        ei_of_chunk.append(cur)
        loc_of_chunk.append(loc)
        loc += 1

    nc = bacc.Bacc("TRN2", target_bir_lowering=False)

    xtp_d = nc.dram_tensor("xtp", [P, xlen], f16, kind="ExternalInput")
    wst_d = nc.dram_tensor("wst", [E, DCH, P, 2 * ISH], f16, kind="ExternalInput")
    w2st_d = nc.dram_tensor("w2st", [E, ICH, P, D], f16, kind="ExternalInput")
    # partial output, d-major transposed: out[p, c, t] = partial[t, c*128 + p]
    out_d = nc.dram_tensor("out", [P, DCH, ntok], f16, kind="ExternalOutput")

    with tile.TileContext(nc) as tc:
        with (
            tc.tile_pool(name="wpool", bufs=32) as wpool,
            tc.tile_pool(name="w2pool", bufs=8) as w2pool,
            tc.tile_pool(name="xpool", bufs=3) as xpool,
            tc.tile_pool(name="spool", bufs=3) as spool,
            tc.tile_pool(name="hpool", bufs=2) as hpool,
            tc.tile_pool(name="opool", bufs=2) as opool,
            tc.tile_pool(name="ph", bufs=5, space="PSUM") as ph_pool,
            tc.tile_pool(name="po", bufs=3, space="PSUM") as po_pool,
        ):
            wtiles: dict = {}
            w2tiles: dict = {}

            def wst_thunk(e, dc):
                def run():
                    t = wpool.tile([P, 2 * ISH], f16, tag="wst", name=f"wst{e}_{dc}")
                    nc.sync.dma_start(t[:], wst_d[e, dc])
                    wtiles.setdefault(e, {})[dc] = t

                return run

            def w2_thunk(e, ic):
                def run():
                    t = w2pool.tile([P, D], f16, tag="w2st", name=f"w2st{e}_{ic}")
                    nc.sync.dma_start(t[:], w2st_d[e, ic])
                    w2tiles.setdefault(e, {})[ic] = t

                return run

            # per expert-index: list of 20 weight-DMA thunks (wst dc0..15, w2 ic0..3)
            wthunks = [
                [wst_thunk(e, dc) for dc in range(DCH)]
                + [w2_thunk(e, ic) for ic in range(ICH)]
                for e in experts_used
            ]
            wissued = [0] * len(experts_used)

            def issue_weights(i, upto):
                upto = min(upto, len(wthunks[i]))
                while wissued[i] < upto:
                    wthunks[i][wissued[i]]()
                    wissued[i] += 1

            xts: dict = {}

            def issue_xt(g):
                e, w = chunks[g]
                t = xpool.tile([P, DCH * CHUNK], f16, tag="xt", name=f"xt{g}")
                nc.sync.dma_start(t[:, : DCH * w], xtp_d[:, xoff[g] : xoff[g] + DCH * w])
                xts[g] = t

            # startup: the first chunk's dc0 x-slice and dc0 weight tile go
            # first so the very first matmul only waits ~1us of DMA; the rest
            # of chunk 0's x and weights stream behind them.
            w0 = chunks[0][1]
            xt0 = xpool.tile([P, DCH * CHUNK], f16, tag="xt", name="xt0")
            nc.sync.dma_start(xt0[:, :w0], xtp_d[:, :w0])
            xts[0] = xt0
            issue_weights(0, 1)
            nc.sync.dma_start(xt0[:, w0 : DCH * w0], xtp_d[:, w0 : DCH * w0])
            issue_weights(0, DCH)
            if nchunks > 1:
                issue_xt(1)
            issue_weights(0, DCH + ICH)

            hTs: dict = {}

            def phase1(g):
                e, w = chunks[g]
                xt = xts.pop(g)
                hT = hpool.tile([P, ICH * CHUNK], f16, tag="hT", name=f"hT{g}")
                for ic in range(ICH):
                    pg = ph_pool.tile([P, CHUNK], mybir.dt.float32, tag="ph", name=f"pg{g}_{ic}")
                    pu = ph_pool.tile([P, CHUNK], mybir.dt.float32, tag="ph", name=f"pu{g}_{ic}")
                    for dc in range(DCH):
                        wt = wtiles[e][dc]
                        xs = xt[:, dc * w : (dc + 1) * w]
                        nc.tensor.matmul(
                            pg[:, :w],
                            wt[:, ic * P : (ic + 1) * P],
                            xs,
                            start=(dc == 0),
                            stop=(dc == DCH - 1),
                        )
                        nc.tensor.matmul(
                            pu[:, :w],
                            wt[:, ISH + ic * P : ISH + (ic + 1) * P],
                            xs,
                            start=(dc == 0),
                            stop=(dc == DCH - 1),
                        )
                    sg = spool.tile([P, CHUNK], f16, tag="sg", name=f"sg{g}_{ic}")
                    nc.scalar.activation(
                        sg[:, :w], pg[:, :w], mybir.ActivationFunctionType.Silu
                    )
                    nc.vector.tensor_mul(
                        hT[:, ic * CHUNK : ic * CHUNK + w], sg[:, :w], pu[:, :w]
                    )
                hTs[g] = hT

            def phase1_dc_outer(g):
                # chunk-0 variant: all 8 accumulation groups live at once
                # (5 ph banks + 3 po banks) so the PE can consume each weight
                # tile the moment its DMA lands, with no ic-sweep re-reads.
                e, w = chunks[g]
                xt = xts.pop(g)
                hT = hpool.tile([P, ICH * CHUNK], f16, tag="hT", name=f"hT{g}")
                pgs = [
                    ph_pool.tile([P, CHUNK], mybir.dt.float32, tag="ph", name=f"pg{g}_{ic}")
                    for ic in range(ICH)
                ]
                pus = [ph_pool.tile([P, CHUNK], mybir.dt.float32, tag="ph", name=f"pu{g}_0")] + [
                    po_pool.tile([P, CHUNK], mybir.dt.float32, tag="po", name=f"pu{g}_{ic}")
                    for ic in range(1, ICH)
                ]
                for dc in range(DCH):
                    wt = wtiles[e][dc]
                    xs = xt[:, dc * w : (dc + 1) * w]
                    for ic in range(ICH):
                        nc.tensor.matmul(
                            pgs[ic][:, :w],
                            wt[:, ic * P : (ic + 1) * P],
                            xs,
                            start=(dc == 0),
                            stop=(dc == DCH - 1),
                        )
                        nc.tensor.matmul(
                            pus[ic][:, :w],
                            wt[:, ISH + ic * P : ISH + (ic + 1) * P],
                            xs,
                            start=(dc == 0),
                            stop=(dc == DCH - 1),
                        )
                for ic in range(ICH):
                    sg = spool.tile([P, CHUNK], f16, tag="sg", name=f"sg{g}_{ic}")
                    nc.scalar.activation(
                        sg[:, :w], pgs[ic][:, :w], mybir.ActivationFunctionType.Silu
                    )
                    nc.vector.tensor_mul(
                        hT[:, ic * CHUNK : ic * CHUNK + w], sg[:, :w], pus[ic][:, :w]
                    )
                hTs[g] = hT

            def down(g):
                e, w = chunks[g]
                hT = hTs.pop(g)
                osb = opool.tile([P, DCH, CHUNK], f16, tag="osb", name=f"osb{g}")
                for dc in range(DCH):
                    po = po_pool.tile([P, CHUNK], mybir.dt.float32, tag="po", name=f"po{g}_{dc}")
                    for ic in range(ICH):
                        nc.tensor.matmul(
                            po[:, :w],
                            w2tiles[e][ic][:, dc * P : (dc + 1) * P],
                            hT[:, ic * CHUNK : ic * CHUNK + w],
                            start=(ic == 0),
                            stop=(ic == ICH - 1),
                        )
                    # evacuate PSUM -> SBUF fp16 (plain copy; combine weights
                    # are applied on the host). Split ACT/DVE.
                    if dc % 2 == 0:
                        nc.scalar.activation(
                            osb[:, dc, :w], po[:, :w], mybir.ActivationFunctionType.Copy
                        )
                    else:
                        nc.vector.tensor_copy(osb[:, dc, :w], po[:, :w])
                # output store on the ACT queue (after its own evac copies):
                # never blocks the SP input-load queue
                base = tokbase[g]
                nc.scalar.dma_start(
                    out_d[:, :, base : base + w], osb[:, :, :w]
                )

            for g in range(nchunks):
                if g + 2 < nchunks:
                    issue_xt(g + 2)
                # prefetch next expert's weights, paced across this expert's chunks
                i = ei_of_chunk[g]
                if i + 1 < len(experts_used):
                    m = nch_of[experts_used[i]]
                    j = loc_of_chunk[g]
                    issue_weights(i + 1, math.ceil(20 * (j + 1) / m))
                if g == 0:
                    phase1_dc_outer(g)
                else:
                    phase1(g)
                if g > 0:
                    down(g - 1)
            down(nchunks - 1)

    nc.compile()
    return nc


def _prepare(hidden_states, router_w, ws, w2s):
    """Host-side routing, packing, transposes, fp16 casts. Returns
    (chunks, ntok, pos, topw, shared inputs dict, per-core weight arrays)."""
    x = np.asarray(hidden_states, dtype=np.float32).reshape(T, D)
    router_w = np.asarray(router_w, dtype=np.float32)
    ws = np.asarray(ws, dtype=np.float32)
    w2s = np.asarray(w2s, dtype=np.float32)

    top1, top2, w1, w2 = _host_router(x, router_w)

    # per-expert token lists
    toks: list[list[int]] = [[] for _ in range(E)]
    for ti in (top1, top2):
        for t in range(T):
            toks[int(ti[t])].append(t)

    order = [e for e in range(E) if len(toks[e]) > 0]

    # exact packing: per expert, ceil(n/CHUNK) near-equal chunks (no tiny
    # tail chunks, whose fixed overheads would stall the PE)
    chunks: list[tuple[int, int]] = []
    perm: list[int] = []
    pos = np.zeros((TOPK, T), dtype=np.int64)
    seen: dict[int, int] = {}
    for e in order:
        n = len(toks[e])
        base = len(perm)
        for j, t in enumerate(toks[e]):
            k = seen.get(t, 0)
            pos[k, t] = base + j
            seen[t] = k + 1
        perm.extend(toks[e])
        parts = math.ceil(n / CHUNK)
        q, r = divmod(n, parts)
        chunks.extend([(e, q + 1)] * r)
        chunks.extend([(e, q)] * (parts - r))
    ntok = len(perm)
    perm_a = np.asarray(perm, dtype=np.int64)

    # NOTE: pos[k, t] maps (token, k-th choice) -> packed row, but the k-th
    # append pass order is (top1, top2); seen[] ensures pos rows follow that.
    topw = np.stack([w1, w2], axis=0)  # [2, T] renormalized weights

    # packed-transposed tokens, fp16, chunk-contiguous per partition:
    # per chunk (w tokens): xtp[p, off + dc*w + j] = x[perm[tb + j], dc*128 + p]
    xb = x[perm_a].astype(np.float16)  # [ntok, D]
    xlen = DCH * ntok
    xtp = np.empty((P, xlen), dtype=np.float16)
    tb = xo = 0
    for _, w in chunks:
        blk = xb[tb : tb + w].reshape(w, DCH, P).transpose(2, 1, 0).reshape(P, DCH * w)
        xtp[:, xo : xo + DCH * w] = blk
        tb += w
        xo += DCH * w

    # per-core weights (fp16)
    wst_all = []
    w2st_all = []
    gate = ws[:, :I, :]  # [E, I, D]
    up = ws[:, I:, :]
    for c in range(NCORES):
        lo, hi = c * ISH, (c + 1) * ISH
        # [E, DCH, P, 2*ISH]: [.., d-part, gate(ISH)||up(ISH)]
        g = gate[:, lo:hi, :].reshape(E, ISH, DCH, P).transpose(0, 2, 3, 1)
        u = up[:, lo:hi, :].reshape(E, ISH, DCH, P).transpose(0, 2, 3, 1)
        wst = np.concatenate([g, u], axis=3)
        wst_all.append(np.ascontiguousarray(wst, dtype=np.float16))
        # w2s[e] is [D, I]; lhsT tile [ic, p(i), d] = w2s[e, d, lo + ic*128 + p]
        w2t = w2s[:, :, lo:hi].transpose(0, 2, 1).reshape(E, ICH, P, D)
        w2st_all.append(np.ascontiguousarray(w2t, dtype=np.float16))

    shared = {"xtp": xtp}
    return tuple(chunks), ntok, pos, topw, shared, wst_all, w2st_all


def kernel(hidden_states, router_w, ws, w2s):
    from concourse import bass_utils

    hs = np.asarray(hidden_states)
    B, S, _ = hs.shape
    chunks, ntok, pos, topw, shared, wst_all, w2st_all = _prepare(
        hidden_states, router_w, ws, w2s
    )

    if chunks not in _CACHE:
        _CACHE[chunks] = _build_bass(chunks)
    nc = _CACHE[chunks]

    in_maps = [
        {**shared, "wst": wst_all[c], "w2st": w2st_all[c]} for c in range(NCORES)
    ]
    res = bass_utils.run_bass_kernel_spmd(nc, in_maps, core_ids=list(range(NCORES)))
    # host combine: sum the 8 I-shard partials (d-major transposed layout),
    # then scale by the combine weights and gather the two expert
    # contributions per token
    acc = np.zeros((P, DCH, ntok), dtype=np.float32)
    for c in range(NCORES):
        acc += res.results[c]["out"].astype(np.float32)
    packed = acc.transpose(2, 1, 0).reshape(ntok, D)  # [t, dc*128+p]
    out = topw[0][:, None] * packed[pos[0]] + topw[1][:, None] * packed[pos[1]]
    return out.reshape(B, S, D).astype(np.float32)


# revision 14
# speedup vs baseline: 1.3482x; 1.0037x over previous
"""DBRX MoE experts kernel for Trainium2 (8 NeuronCores).

Strategy (v3):
  - Router (logits -> softmax -> top-2 -> renormalize) computed on host in numpy
    (0.01% of FLOPs); it determines the token->expert dispatch, i.e. the sharding.
  - Tensor-parallel over the FFN intermediate dim across 8 cores: core c owns
    I-slice [c*512:(c+1)*512) of every expert (ws rows for gate and up, w2s cols).
  - Top-2 sparsity: tokens are packed per expert EXACTLY (no padding): per
    expert, chunks of <=256 tokens (full 256-chunks plus one ragged tail).
  - All matmul operands are fp16 (1.0 PE cycles/row at any free size, half the
    HBM traffic of fp32); accumulation is fp32 in PSUM. Every matmul streams
    N=w tokens (gate/up: [128i x w], down: [128d x w]), so PE time is exactly
    proportional to routed tokens: no padding waste anywhere.
  - The combine weights are applied on the HOST during the final gather
    (out[t] = cw0[t]*packed[pos0[t]] + cw1[t]*packed[pos1[t]]), so the device
    writes unscaled partials and PSUM evacuation is a plain copy.
  - No on-device collective: each core writes its partial output in a
    d-major transposed layout [128, 16, ntok] (fp16); the host sums the 8
    partials, transposes, scales and gathers.
  - Pipelining: per chunk c the program issues P1(c) (gate/up+SwiGLU) then
    Down(c-1), so the PE never waits on the DVE/ACT producing h. Input loads
    (x chunks, weights) ride the SP sequencer queue with weights prefetched
    one expert ahead; output stores ride the ACT queue (issued after that
    chunk's evacuation copies), so they never stall input loads.
"""

import math

import numpy as np

T = 4096
D = 2048
E = 8
I = 4096
TOPK = 2
NCORES = 8
ISH = I // NCORES  # 512, per-core I shard
P = 128
DCH = D // P  # 16 d-chunks
ICH = ISH // P  # 4 i-chunks
CHUNK = 512  # max token chunk (PSUM bank holds 512 fp32 per partition)


def _host_router(x, router_w):
    """Replicate reference routing in numpy (fp32)."""
    logits = (x.astype(np.float64) @ router_w.astype(np.float64).T).astype(np.float32)
    m = logits.max(axis=-1, keepdims=True)
    ex = np.exp((logits - m).astype(np.float32))
    probs = ex / ex.sum(axis=-1, keepdims=True)
    # top-2, ties to lower index (matches jax.lax.top_k)
    top1 = probs.argmax(axis=-1)
    p = probs.copy()
    p[np.arange(T), top1] = -1.0
    top2 = p.argmax(axis=-1)
    w1 = probs[np.arange(T), top1]
    w2 = probs[np.arange(T), top2]
    s = w1 + w2
    return top1.astype(np.int64), top2.astype(np.int64), (w1 / s).astype(np.float32), (w2 / s).astype(np.float32)


_CACHE: dict = {}


def _build_bass(chunks):
    """Build the 8-core SPMD Bass program.

    chunks: tuple of (expert, width) in packed-token order; widths sum to T*TOPK.
    """
    import concourse.bacc as bacc
    import concourse.mybir as mybir
    import concourse.tile as tile

    f16 = mybir.dt.float16

    nchunks = len(chunks)
    xlen = sum(DCH * w for _, w in chunks)
    ntok = sum(w for _, w in chunks)

    # per-chunk packed-token base and x offset
    tokbase = []
    xoff = []
    tb = xo = 0
    for _, w in chunks:
        tokbase.append(tb)
        xoff.append(xo)
        tb += w
        xo += DCH * w

    # expert schedule: unique experts in chunk order, with local chunk counts
    experts_used = []
    for e, _ in chunks:
        if not experts_used or experts_used[-1] != e:
            experts_used.append(e)
    nch_of = {e: sum(1 for ee, _ in chunks if ee == e) for e in experts_used}
    ei_of_chunk = []  # expert-INDEX per global chunk
    loc_of_chunk = []  # local chunk index within its expert
    cur = -1
    loc = 0
    for e, _ in chunks:
        if cur == -1 or experts_used[cur] != e:
            cur += 1
            loc = 0
        ei_of_chunk.append(cur)
        loc_of_chunk.append(loc)
        loc += 1

    nc = bacc.Bacc("TRN2", target_bir_lowering=False)

    xtp_d = nc.dram_tensor("xtp", [P, xlen], f16, kind="ExternalInput")
    wst_d = nc.dram_tensor("wst", [E, DCH, P, 2 * ISH], f16, kind="ExternalInput")
    w2st_d = nc.dram_tensor("w2st", [E, ICH, P, D], f16, kind="ExternalInput")
    # partial output, d-major transposed: out[p, c, t] = partial[t, c*128 + p]
    out_d = nc.dram_tensor("out", [P, DCH, ntok], f16, kind="ExternalOutput")

    with tile.TileContext(nc) as tc:
        with (
            tc.tile_pool(name="wpool", bufs=32) as wpool,
            tc.tile_pool(name="w2pool", bufs=8) as w2pool,
            tc.tile_pool(name="xpool", bufs=3) as xpool,
            tc.tile_pool(name="spool", bufs=3) as spool,
            tc.tile_pool(name="hpool", bufs=2) as hpool,
            tc.tile_pool(name="opool", bufs=2) as opool,
            tc.tile_pool(name="ph", bufs=5, space="PSUM") as ph_pool,
            tc.tile_pool(name="po", bufs=3, space="PSUM") as po_pool,
        ):
            wtiles: dict = {}
            w2tiles: dict = {}

            def wst_thunk(e, dc):
                def run():
                    t = wpool.tile([P, 2 * ISH], f16, tag="wst", name=f"wst{e}_{dc}")
                    nc.sync.dma_start(t[:], wst_d[e, dc])
                    wtiles.setdefault(e, {})[dc] = t

                return run

            def w2_thunk(e, ic):
                def run():
                    t = w2pool.tile([P, D], f16, tag="w2st", name=f"w2st{e}_{ic}")
                    nc.sync.dma_start(t[:], w2st_d[e, ic])
                    w2tiles.setdefault(e, {})[ic] = t

                return run

            # per expert-index: list of 20 weight-DMA thunks (wst dc0..15, w2 ic0..3)
            wthunks = [
                [wst_thunk(e, dc) for dc in range(DCH)]
                + [w2_thunk(e, ic) for ic in range(ICH)]
                for e in experts_used
            ]
            wissued = [0] * len(experts_used)

            def issue_weights(i, upto):
                upto = min(upto, len(wthunks[i]))
                while wissued[i] < upto:
                    wthunks[i][wissued[i]]()
                    wissued[i] += 1

            xts: dict = {}

            def issue_xt(g):
                e, w = chunks[g]
                t = xpool.tile([P, DCH * CHUNK], f16, tag="xt", name=f"xt{g}")
                nc.sync.dma_start(t[:, : DCH * w], xtp_d[:, xoff[g] : xoff[g] + DCH * w])
                xts[g] = t

            # startup: interleave per-dc slices of the first x chunk with the
            # first weight tiles so the PE's dc-outer sweep of chunk 0 starts
            # after ~2us of DMA and never starves thereafter.
            w0 = chunks[0][1]
            xt0 = xpool.tile([P, DCH * CHUNK], f16, tag="xt", name="xt0")
            xts[0] = xt0
            NPIECE = 4
            for dc in range(NPIECE):
                nc.sync.dma_start(
                    xt0[:, dc * w0 : (dc + 1) * w0],
                    xtp_d[:, dc * w0 : (dc + 1) * w0],
                )
                issue_weights(0, dc + 1)
            nc.sync.dma_start(
                xt0[:, NPIECE * w0 : DCH * w0], xtp_d[:, NPIECE * w0 : DCH * w0]
            )
            issue_weights(0, DCH)
            if nchunks > 1:
                issue_xt(1)
            issue_weights(0, DCH + ICH)

            hTs: dict = {}

            def phase1(g):
                e, w = chunks[g]
                xt = xts.pop(g)
                hT = hpool.tile([P, ICH * CHUNK], f16, tag="hT", name=f"hT{g}")
                for ic in range(ICH):
                    pg = ph_pool.tile([P, CHUNK], mybir.dt.float32, tag="ph", name=f"pg{g}_{ic}")
                    pu = ph_pool.tile([P, CHUNK], mybir.dt.float32, tag="ph", name=f"pu{g}_{ic}")
                    for dc in range(DCH):
                        wt = wtiles[e][dc]
                        xs = xt[:, dc * w : (dc + 1) * w]
                        nc.tensor.matmul(
                            pg[:, :w],
                            wt[:, ic * P : (ic + 1) * P],
                            xs,
                            start=(dc == 0),
                            stop=(dc == DCH - 1),
                        )
                        nc.tensor.matmul(
                            pu[:, :w],
                            wt[:, ISH + ic * P : ISH + (ic + 1) * P],
                            xs,
                            start=(dc == 0),
                            stop=(dc == DCH - 1),
                        )
                    sg = spool.tile([P, CHUNK], f16, tag="sg", name=f"sg{g}_{ic}")
                    nc.scalar.activation(
                        sg[:, :w], pg[:, :w], mybir.ActivationFunctionType.Silu
                    )
                    nc.vector.tensor_mul(
                        hT[:, ic * CHUNK : ic * CHUNK + w], sg[:, :w], pu[:, :w]
                    )
                hTs[g] = hT

            def phase1_dc_outer(g):
                # chunk-0 variant: all 8 accumulation groups live at once
                # (5 ph banks + 3 po banks) so the PE can consume each weight
                # tile the moment its DMA lands, with no ic-sweep re-reads.
                e, w = chunks[g]
                xt = xts.pop(g)
                hT = hpool.tile([P, ICH * CHUNK], f16, tag="hT", name=f"hT{g}")
                pgs = [
                    ph_pool.tile([P, CHUNK], mybir.dt.float32, tag="ph", name=f"pg{g}_{ic}")
                    for ic in range(ICH)
                ]
                pus = [ph_pool.tile([P, CHUNK], mybir.dt.float32, tag="ph", name=f"pu{g}_0")] + [
                    po_pool.tile([P, CHUNK], mybir.dt.float32, tag="po", name=f"pu{g}_{ic}")
                    for ic in range(1, ICH)
                ]
                for dc in range(DCH):
                    wt = wtiles[e][dc]
                    xs = xt[:, dc * w : (dc + 1) * w]
                    for ic in range(ICH):
                        nc.tensor.matmul(
                            pgs[ic][:, :w],
                            wt[:, ic * P : (ic + 1) * P],
                            xs,
                            start=(dc == 0),
                            stop=(dc == DCH - 1),
                        )
                        nc.tensor.matmul(
                            pus[ic][:, :w],
                            wt[:, ISH + ic * P : ISH + (ic + 1) * P],
                            xs,
                            start=(dc == 0),
                            stop=(dc == DCH - 1),
                        )
                for ic in range(ICH):
                    sg = spool.tile([P, CHUNK], f16, tag="sg", name=f"sg{g}_{ic}")
                    nc.scalar.activation(
                        sg[:, :w], pgs[ic][:, :w], mybir.ActivationFunctionType.Silu
                    )
                    nc.vector.tensor_mul(
                        hT[:, ic * CHUNK : ic * CHUNK + w], sg[:, :w], pus[ic][:, :w]
                    )
                hTs[g] = hT

            def down(g, fine_store=False):
                e, w = chunks[g]
                hT = hTs.pop(g)
                base = tokbase[g]
                osb = opool.tile([P, DCH, CHUNK], f16, tag="osb", name=f"osb{g}")
                for dc in range(DCH):
                    po = po_pool.tile([P, CHUNK], mybir.dt.float32, tag="po", name=f"po{g}_{dc}")
                    for ic in range(ICH):
                        nc.tensor.matmul(
                            po[:, :w],
                            w2tiles[e][ic][:, dc * P : (dc + 1) * P],
                            hT[:, ic * CHUNK : ic * CHUNK + w],
                            start=(ic == 0),
                            stop=(ic == ICH - 1),
                        )
                    # evacuate PSUM -> SBUF fp16 (plain copy; combine weights
                    # are applied on the host). Split ACT/DVE.
                    if dc % 2 == 0:
                        nc.scalar.activation(
                            osb[:, dc, :w], po[:, :w], mybir.ActivationFunctionType.Copy
                        )
                    else:
                        nc.vector.tensor_copy(osb[:, dc, :w], po[:, :w])
                    if fine_store:
                        # final chunk: store per-dc so the transfers overlap
                        # the remaining Down matmuls instead of sitting wholly
                        # in the program tail
                        nc.scalar.dma_start(
                            out_d[:, dc, base : base + w], osb[:, dc, :w]
                        )
                if not fine_store:
                    # output store on the ACT queue (after its own evac
                    # copies): never blocks the SP input-load queue
                    nc.scalar.dma_start(
                        out_d[:, :, base : base + w], osb[:, :, :w]
                    )

            for g in range(nchunks):
                if g + 2 < nchunks:
                    issue_xt(g + 2)
                # prefetch next expert's weights, paced across this expert's chunks
                i = ei_of_chunk[g]
                if i + 1 < len(experts_used):
                    m = nch_of[experts_used[i]]
                    j = loc_of_chunk[g]
                    issue_weights(i + 1, math.ceil(20 * (j + 1) / m))
                if g == 0:
                    phase1_dc_outer(g)
                else:
                    phase1(g)
                if g > 0:
                    down(g - 1)
            down(nchunks - 1, fine_store=True)

    nc.compile()
    return nc


def _prepare(hidden_states, router_w, ws, w2s):
    """Host-side routing, packing, transposes, fp16 casts. Returns
    (chunks, ntok, pos, topw, shared inputs dict, per-core weight arrays)."""
    x = np.asarray(hidden_states, dtype=np.float32).reshape(T, D)
    router_w = np.asarray(router_w, dtype=np.float32)
    ws = np.asarray(ws, dtype=np.float32)
    w2s = np.asarray(w2s, dtype=np.float32)

    top1, top2, w1, w2 = _host_router(x, router_w)

    # per-expert token lists
    toks: list[list[int]] = [[] for _ in range(E)]
    for ti in (top1, top2):
        for t in range(T):
            toks[int(ti[t])].append(t)

    order = [e for e in range(E) if len(toks[e]) > 0]

    # exact packing: per expert, ceil(n/CHUNK) near-equal chunks (no tiny
    # tail chunks, whose fixed overheads would stall the PE)
    chunks: list[tuple[int, int]] = []
    perm: list[int] = []
    pos = np.zeros((TOPK, T), dtype=np.int64)
    seen: dict[int, int] = {}
    for e in order:
        n = len(toks[e])
        base = len(perm)
        for j, t in enumerate(toks[e]):
            k = seen.get(t, 0)
            pos[k, t] = base + j
            seen[t] = k + 1
        perm.extend(toks[e])
        parts = math.ceil(n / CHUNK)
        q, r = divmod(n, parts)
        chunks.extend([(e, q + 1)] * r)
        chunks.extend([(e, q)] * (parts - r))
    ntok = len(perm)
    perm_a = np.asarray(perm, dtype=np.int64)

    # NOTE: pos[k, t] maps (token, k-th choice) -> packed row, but the k-th
    # append pass order is (top1, top2); seen[] ensures pos rows follow that.
    topw = np.stack([w1, w2], axis=0)  # [2, T] renormalized weights

    # packed-transposed tokens, fp16, chunk-contiguous per partition:
    # per chunk (w tokens): xtp[p, off + dc*w + j] = x[perm[tb + j], dc*128 + p]
    xb = x[perm_a].astype(np.float16)  # [ntok, D]
    xlen = DCH * ntok
    xtp = np.empty((P, xlen), dtype=np.float16)
    tb = xo = 0
    for _, w in chunks:
        blk = xb[tb : tb + w].reshape(w, DCH, P).transpose(2, 1, 0).reshape(P, DCH * w)
        xtp[:, xo : xo + DCH * w] = blk
        tb += w
        xo += DCH * w

    # per-core weights (fp16)
    wst_all = []
    w2st_all = []
    gate = ws[:, :I, :]  # [E, I, D]
    up = ws[:, I:, :]
    for c in range(NCORES):
        lo, hi = c * ISH, (c + 1) * ISH
        # [E, DCH, P, 2*ISH]: [.., d-part, gate(ISH)||up(ISH)]
        g = gate[:, lo:hi, :].reshape(E, ISH, DCH, P).transpose(0, 2, 3, 1)
        u = up[:, lo:hi, :].reshape(E, ISH, DCH, P).transpose(0, 2, 3, 1)
        wst = np.concatenate([g, u], axis=3)
        wst_all.append(np.ascontiguousarray(wst, dtype=np.float16))
        # w2s[e] is [D, I]; lhsT tile [ic, p(i), d] = w2s[e, d, lo + ic*128 + p]
        w2t = w2s[:, :, lo:hi].transpose(0, 2, 1).reshape(E, ICH, P, D)
        w2st_all.append(np.ascontiguousarray(w2t, dtype=np.float16))

    shared = {"xtp": xtp}
    return tuple(chunks), ntok, pos, topw, shared, wst_all, w2st_all


def kernel(hidden_states, router_w, ws, w2s):
    from concourse import bass_utils

    hs = np.asarray(hidden_states)
    B, S, _ = hs.shape
    chunks, ntok, pos, topw, shared, wst_all, w2st_all = _prepare(
        hidden_states, router_w, ws, w2s
    )

    if chunks not in _CACHE:
        _CACHE[chunks] = _build_bass(chunks)
    nc = _CACHE[chunks]

    in_maps = [
        {**shared, "wst": wst_all[c], "w2st": w2st_all[c]} for c in range(NCORES)
    ]
    res = bass_utils.run_bass_kernel_spmd(nc, in_maps, core_ids=list(range(NCORES)))
    # host combine: sum the 8 I-shard partials (d-major transposed layout),
    # then scale by the combine weights and gather the two expert
    # contributions per token
    acc = np.zeros((P, DCH, ntok), dtype=np.float32)
    for c in range(NCORES):
        acc += res.results[c]["out"].astype(np.float32)
    packed = acc.transpose(2, 1, 0).reshape(ntok, D)  # [t, dc*128+p]
    out = topw[0][:, None] * packed[pos[0]] + topw[1][:, None] * packed[pos[1]]
    return out.reshape(B, S, D).astype(np.float32)


# revision 19
# speedup vs baseline: 1.3594x; 1.0083x over previous
"""DBRX MoE experts kernel for Trainium2 (8 NeuronCores).

Strategy (v3):
  - Router (logits -> softmax -> top-2 -> renormalize) computed on host in numpy
    (0.01% of FLOPs); it determines the token->expert dispatch, i.e. the sharding.
  - Tensor-parallel over the FFN intermediate dim across 8 cores: core c owns
    I-slice [c*512:(c+1)*512) of every expert (ws rows for gate and up, w2s cols).
  - Top-2 sparsity: tokens are packed per expert EXACTLY (no padding): per
    expert, chunks of <=256 tokens (full 256-chunks plus one ragged tail).
  - All matmul operands are fp16 (1.0 PE cycles/row at any free size, half the
    HBM traffic of fp32); accumulation is fp32 in PSUM. Every matmul streams
    N=w tokens (gate/up: [128i x w], down: [128d x w]), so PE time is exactly
    proportional to routed tokens: no padding waste anywhere.
  - The combine weights are applied on the HOST during the final gather
    (out[t] = cw0[t]*packed[pos0[t]] + cw1[t]*packed[pos1[t]]), so the device
    writes unscaled partials and PSUM evacuation is a plain copy.
  - No on-device collective: each core writes its partial output in a
    d-major transposed layout [128, 16, ntok] (fp16); the host sums the 8
    partials, transposes, scales and gathers.
  - Pipelining: per chunk c the program issues P1(c) (gate/up+SwiGLU) then
    Down(c-1), so the PE never waits on the DVE/ACT producing h. Input loads
    (x chunks, weights) ride the SP sequencer queue with weights prefetched
    one expert ahead; output stores ride the ACT queue (issued after that
    chunk's evacuation copies), so they never stall input loads.
"""

import math

import numpy as np

T = 4096
D = 2048
E = 8
I = 4096
TOPK = 2
NCORES = 8
ISH = I // NCORES  # 512, per-core I shard
P = 128
DCH = D // P  # 16 d-chunks
ICH = ISH // P  # 4 i-chunks
CHUNK = 512  # max token chunk (PSUM bank holds 512 fp32 per partition)


def _host_router(x, router_w):
    """Replicate reference routing in numpy (fp32)."""
    logits = (x.astype(np.float64) @ router_w.astype(np.float64).T).astype(np.float32)
    m = logits.max(axis=-1, keepdims=True)
    ex = np.exp((logits - m).astype(np.float32))
    probs = ex / ex.sum(axis=-1, keepdims=True)
    # top-2, ties to lower index (matches jax.lax.top_k)
    top1 = probs.argmax(axis=-1)
    p = probs.copy()
    p[np.arange(T), top1] = -1.0
    top2 = p.argmax(axis=-1)
    w1 = probs[np.arange(T), top1]
    w2 = probs[np.arange(T), top2]
    s = w1 + w2
    return top1.astype(np.int64), top2.astype(np.int64), (w1 / s).astype(np.float32), (w2 / s).astype(np.float32)


_CACHE: dict = {}


def _build_bass(chunks):
    """Build the 8-core SPMD Bass program.

    chunks: tuple of (expert, width) in packed-token order; widths sum to T*TOPK.
    """
    import concourse.bacc as bacc
    import concourse.mybir as mybir
    import concourse.tile as tile

    f16 = mybir.dt.float16

    nchunks = len(chunks)
    xlen = sum(DCH * w for _, w in chunks)
    ntok = sum(w for _, w in chunks)

    # per-chunk packed-token base and x offset
    tokbase = []
    xoff = []
    tb = xo = 0
    for _, w in chunks:
        tokbase.append(tb)
        xoff.append(xo)
        tb += w
        xo += DCH * w

    # expert schedule: unique experts in chunk order, with local chunk counts
    experts_used = []
    for e, _ in chunks:
        if not experts_used or experts_used[-1] != e:
            experts_used.append(e)
    nch_of = {e: sum(1 for ee, _ in chunks if ee == e) for e in experts_used}
    ei_of_chunk = []  # expert-INDEX per global chunk
    loc_of_chunk = []  # local chunk index within its expert
    cur = -1
    loc = 0
    for e, _ in chunks:
        if cur == -1 or experts_used[cur] != e:
            cur += 1
            loc = 0
        ei_of_chunk.append(cur)
        loc_of_chunk.append(loc)
        loc += 1

    nc = bacc.Bacc("TRN2", target_bir_lowering=False)

    xtp_d = nc.dram_tensor("xtp", [P, xlen], f16, kind="ExternalInput")
    wst_d = nc.dram_tensor("wst", [E, DCH, P, 2 * ISH], f16, kind="ExternalInput")
    w2st_d = nc.dram_tensor("w2st", [E, ICH, P, D], f16, kind="ExternalInput")
    # partial output, d-major transposed: out[p, c, t] = partial[t, c*128 + p]
    out_d = nc.dram_tensor("out", [P, DCH, ntok], f16, kind="ExternalOutput")

    with tile.TileContext(nc) as tc:
        with (
            tc.tile_pool(name="wpool", bufs=32) as wpool,
            tc.tile_pool(name="w2pool", bufs=8) as w2pool,
            tc.tile_pool(name="xpool", bufs=3) as xpool,
            tc.tile_pool(name="spool", bufs=3) as spool,
            tc.tile_pool(name="hpool", bufs=2) as hpool,
            tc.tile_pool(name="opool", bufs=2) as opool,
            tc.tile_pool(name="misc", bufs=1) as mpool,
            tc.tile_pool(name="ph", bufs=5, space="PSUM") as ph_pool,
            tc.tile_pool(name="po", bufs=3, space="PSUM") as po_pool,
        ):
            wtiles: dict = {}
            w2tiles: dict = {}

            def wst_thunk(e, dc):
                def run():
                    t = wpool.tile([P, 2 * ISH], f16, tag="wst", name=f"wst{e}_{dc}")
                    nc.sync.dma_start(t[:], wst_d[e, dc])
                    wtiles.setdefault(e, {})[dc] = t

                return run

            def w2_thunk(e, ic):
                def run():
                    t = w2pool.tile([P, D], f16, tag="w2st", name=f"w2st{e}_{ic}")
                    nc.sync.dma_start(t[:], w2st_d[e, ic])
                    w2tiles.setdefault(e, {})[ic] = t

                return run

            # per expert-index: list of 20 weight-DMA thunks (wst dc0..15, w2 ic0..3)
            wthunks = [
                [wst_thunk(e, dc) for dc in range(DCH)]
                + [w2_thunk(e, ic) for ic in range(ICH)]
                for e in experts_used
            ]
            wissued = [0] * len(experts_used)

            def issue_weights(i, upto):
                upto = min(upto, len(wthunks[i]))
                while wissued[i] < upto:
                    wthunks[i][wissued[i]]()
                    wissued[i] += 1

            xts: dict = {}

            def issue_xt(g):
                e, w = chunks[g]
                t = xpool.tile([P, DCH * CHUNK], f16, tag="xt", name=f"xt{g}")
                nc.sync.dma_start(t[:, : DCH * w], xtp_d[:, xoff[g] : xoff[g] + DCH * w])
                xts[g] = t

            # PE warmup: dummy matmuls on a zeroed tile while the first DMAs
            # are in flight, so the PE clock is at full speed (the cost
            # model's p-state ramp needs ~3us of continuous PE activity) when
            # the first real matmul issues.
            wz = mpool.tile([P, 64], f16, name="wz")
            nc.vector.memset(wz[:], 0)
            pw = ph_pool.tile([P, CHUNK], mybir.dt.float32, tag="ph", name="pwarm")
            for k in range(70):
                nc.tensor.matmul(pw[:64, :64], wz[:], wz[:], start=True, stop=True)

            # startup: interleave per-dc slices of the first x chunk with the
            # first weight tiles so the PE's dc-outer sweep of chunk 0 starts
            # after ~2us of DMA and never starves thereafter.
            w0 = chunks[0][1]
            xt0 = xpool.tile([P, DCH * CHUNK], f16, tag="xt", name="xt0")
            xts[0] = xt0
            NPIECE = 4
            for dc in range(NPIECE):
                nc.sync.dma_start(
                    xt0[:, dc * w0 : (dc + 1) * w0],
                    xtp_d[:, dc * w0 : (dc + 1) * w0],
                )
                issue_weights(0, dc + 1)
            nc.sync.dma_start(
                xt0[:, NPIECE * w0 : DCH * w0], xtp_d[:, NPIECE * w0 : DCH * w0]
            )
            issue_weights(0, DCH)
            if nchunks > 1:
                issue_xt(1)
            issue_weights(0, DCH + ICH)

            hTs: dict = {}

            def phase1(g):
                e, w = chunks[g]
                xt = xts.pop(g)
                hT = hpool.tile([P, ICH * CHUNK], f16, tag="hT", name=f"hT{g}")
                for ic in range(ICH):
                    pg = ph_pool.tile([P, CHUNK], mybir.dt.float32, tag="ph", name=f"pg{g}_{ic}")
                    pu = ph_pool.tile([P, CHUNK], mybir.dt.float32, tag="ph", name=f"pu{g}_{ic}")
                    for dc in range(DCH):
                        wt = wtiles[e][dc]
                        xs = xt[:, dc * w : (dc + 1) * w]
                        nc.tensor.matmul(
                            pg[:, :w],
                            wt[:, ic * P : (ic + 1) * P],
                            xs,
                            start=(dc == 0),
                            stop=(dc == DCH - 1),
                        )
                        nc.tensor.matmul(
                            pu[:, :w],
                            wt[:, ISH + ic * P : ISH + (ic + 1) * P],
                            xs,
                            start=(dc == 0),
                            stop=(dc == DCH - 1),
                        )
                    sg = spool.tile([P, CHUNK], f16, tag="sg", name=f"sg{g}_{ic}")
                    nc.scalar.activation(
                        sg[:, :w], pg[:, :w], mybir.ActivationFunctionType.Silu
                    )
                    nc.vector.tensor_mul(
                        hT[:, ic * CHUNK : ic * CHUNK + w], sg[:, :w], pu[:, :w]
                    )
                hTs[g] = hT

            def phase1_dc_outer(g):
                # chunk-0 variant: all 8 accumulation groups live at once
                # (5 ph banks + 3 po banks) so the PE can consume each weight
                # tile the moment its DMA lands, with no ic-sweep re-reads.
                e, w = chunks[g]
                xt = xts.pop(g)
                hT = hpool.tile([P, ICH * CHUNK], f16, tag="hT", name=f"hT{g}")
                pgs = [
                    ph_pool.tile([P, CHUNK], mybir.dt.float32, tag="ph", name=f"pg{g}_{ic}")
                    for ic in range(ICH)
                ]
                pus = [ph_pool.tile([P, CHUNK], mybir.dt.float32, tag="ph", name=f"pu{g}_0")] + [
                    po_pool.tile([P, CHUNK], mybir.dt.float32, tag="po", name=f"pu{g}_{ic}")
                    for ic in range(1, ICH)
                ]
                for dc in range(DCH):
                    wt = wtiles[e][dc]
                    xs = xt[:, dc * w : (dc + 1) * w]
                    for ic in range(ICH):
                        nc.tensor.matmul(
                            pgs[ic][:, :w],
                            wt[:, ic * P : (ic + 1) * P],
                            xs,
                            start=(dc == 0),
                            stop=(dc == DCH - 1),
                        )
                        nc.tensor.matmul(
                            pus[ic][:, :w],
                            wt[:, ISH + ic * P : ISH + (ic + 1) * P],
                            xs,
                            start=(dc == 0),
                            stop=(dc == DCH - 1),
                        )
                for ic in range(ICH):
                    sg = spool.tile([P, CHUNK], f16, tag="sg", name=f"sg{g}_{ic}")
                    nc.scalar.activation(
                        sg[:, :w], pgs[ic][:, :w], mybir.ActivationFunctionType.Silu
                    )
                    nc.vector.tensor_mul(
                        hT[:, ic * CHUNK : ic * CHUNK + w], sg[:, :w], pus[ic][:, :w]
                    )
                hTs[g] = hT

            def down(g, fine_store=False):
                e, w = chunks[g]
                hT = hTs.pop(g)
                base = tokbase[g]
                osb = opool.tile([P, DCH, CHUNK], f16, tag="osb", name=f"osb{g}")
                for dc in range(DCH):
                    po = po_pool.tile([P, CHUNK], mybir.dt.float32, tag="po", name=f"po{g}_{dc}")
                    for ic in range(ICH):
                        nc.tensor.matmul(
                            po[:, :w],
                            w2tiles[e][ic][:, dc * P : (dc + 1) * P],
                            hT[:, ic * CHUNK : ic * CHUNK + w],
                            start=(ic == 0),
                            stop=(ic == ICH - 1),
                        )
                    # evacuate PSUM -> SBUF fp16 (plain copy; combine weights
                    # are applied on the host). Split ACT/DVE.
                    if dc % 2 == 0:
                        nc.scalar.activation(
                            osb[:, dc, :w], po[:, :w], mybir.ActivationFunctionType.Copy
                        )
                    else:
                        nc.vector.tensor_copy(osb[:, dc, :w], po[:, :w])
                    if fine_store:
                        # final chunk: store per-dc so the transfers overlap
                        # the remaining Down matmuls instead of sitting wholly
                        # in the program tail
                        nc.scalar.dma_start(
                            out_d[:, dc, base : base + w], osb[:, dc, :w]
                        )
                if not fine_store:
                    # output store on the ACT queue (after its own evac
                    # copies): never blocks the SP input-load queue
                    nc.scalar.dma_start(
                        out_d[:, :, base : base + w], osb[:, :, :w]
                    )

            for g in range(nchunks):
                if g + 2 < nchunks:
                    issue_xt(g + 2)
                # prefetch next expert's weights, paced across this expert's chunks
                i = ei_of_chunk[g]
                if i + 1 < len(experts_used):
                    m = nch_of[experts_used[i]]
                    j = loc_of_chunk[g]
                    issue_weights(i + 1, math.ceil(20 * (j + 1) / m))
                if g == 0:
                    phase1_dc_outer(g)
                else:
                    phase1(g)
                if g > 0:
                    down(g - 1)
            down(nchunks - 1, fine_store=True)

    nc.compile()
    return nc


def _prepare(hidden_states, router_w, ws, w2s):
    """Host-side routing, packing, transposes, fp16 casts. Returns
    (chunks, ntok, pos, topw, shared inputs dict, per-core weight arrays)."""
    x = np.asarray(hidden_states, dtype=np.float32).reshape(T, D)
    router_w = np.asarray(router_w, dtype=np.float32)
    ws = np.asarray(ws, dtype=np.float32)
    w2s = np.asarray(w2s, dtype=np.float32)

    top1, top2, w1, w2 = _host_router(x, router_w)

    # per-expert token lists
    toks: list[list[int]] = [[] for _ in range(E)]
    for ti in (top1, top2):
        for t in range(T):
            toks[int(ti[t])].append(t)

    order = [e for e in range(E) if len(toks[e]) > 0]

    # exact packing: per expert, ceil(n/CHUNK) near-equal chunks (no tiny
    # tail chunks, whose fixed overheads would stall the PE)
    chunks: list[tuple[int, int]] = []
    perm: list[int] = []
    pos = np.zeros((TOPK, T), dtype=np.int64)
    seen: dict[int, int] = {}
    for e in order:
        n = len(toks[e])
        base = len(perm)
        for j, t in enumerate(toks[e]):
            k = seen.get(t, 0)
            pos[k, t] = base + j
            seen[t] = k + 1
        perm.extend(toks[e])
        parts = math.ceil(n / CHUNK)
        q, r = divmod(n, parts)
        chunks.extend([(e, q + 1)] * r)
        chunks.extend([(e, q)] * (parts - r))
    ntok = len(perm)
    perm_a = np.asarray(perm, dtype=np.int64)

    # NOTE: pos[k, t] maps (token, k-th choice) -> packed row, but the k-th
    # append pass order is (top1, top2); seen[] ensures pos rows follow that.
    topw = np.stack([w1, w2], axis=0)  # [2, T] renormalized weights

    # packed-transposed tokens, fp16, chunk-contiguous per partition:
    # per chunk (w tokens): xtp[p, off + dc*w + j] = x[perm[tb + j], dc*128 + p]
    xb = x[perm_a].astype(np.float16)  # [ntok, D]
    xlen = DCH * ntok
    xtp = np.empty((P, xlen), dtype=np.float16)
    tb = xo = 0
    for _, w in chunks:
        blk = xb[tb : tb + w].reshape(w, DCH, P).transpose(2, 1, 0).reshape(P, DCH * w)
        xtp[:, xo : xo + DCH * w] = blk
        tb += w
        xo += DCH * w

    # per-core weights (fp16)
    wst_all = []
    w2st_all = []
    gate = ws[:, :I, :]  # [E, I, D]
    up = ws[:, I:, :]
    for c in range(NCORES):
        lo, hi = c * ISH, (c + 1) * ISH
        # [E, DCH, P, 2*ISH]: [.., d-part, gate(ISH)||up(ISH)]
        g = gate[:, lo:hi, :].reshape(E, ISH, DCH, P).transpose(0, 2, 3, 1)
        u = up[:, lo:hi, :].reshape(E, ISH, DCH, P).transpose(0, 2, 3, 1)
        wst = np.concatenate([g, u], axis=3)
        wst_all.append(np.ascontiguousarray(wst, dtype=np.float16))
        # w2s[e] is [D, I]; lhsT tile [ic, p(i), d] = w2s[e, d, lo + ic*128 + p]
        w2t = w2s[:, :, lo:hi].transpose(0, 2, 1).reshape(E, ICH, P, D)
        w2st_all.append(np.ascontiguousarray(w2t, dtype=np.float16))

    shared = {"xtp": xtp}
    return tuple(chunks), ntok, pos, topw, shared, wst_all, w2st_all


def kernel(hidden_states, router_w, ws, w2s):
    from concourse import bass_utils

    hs = np.asarray(hidden_states)
    B, S, _ = hs.shape
    chunks, ntok, pos, topw, shared, wst_all, w2st_all = _prepare(
        hidden_states, router_w, ws, w2s
    )

    if chunks not in _CACHE:
        _CACHE[chunks] = _build_bass(chunks)
    nc = _CACHE[chunks]

    in_maps = [
        {**shared, "wst": wst_all[c], "w2st": w2st_all[c]} for c in range(NCORES)
    ]
    res = bass_utils.run_bass_kernel_spmd(nc, in_maps, core_ids=list(range(NCORES)))
    # host combine: sum the 8 I-shard partials (d-major transposed layout),
    # then scale by the combine weights and gather the two expert
    # contributions per token
    acc = np.zeros((P, DCH, ntok), dtype=np.float32)
    for c in range(NCORES):
        acc += res.results[c]["out"].astype(np.float32)
    packed = acc.transpose(2, 1, 0).reshape(ntok, D)  # [t, dc*128+p]
    out = topw[0][:, None] * packed[pos[0]] + topw[1][:, None] * packed[pos[1]]
    return out.reshape(B, S, D).astype(np.float32)


# revision 21
# speedup vs baseline: 1.3631x; 1.0027x over previous
"""DBRX MoE experts kernel for Trainium2 (8 NeuronCores).

Strategy (v3):
  - Router (logits -> softmax -> top-2 -> renormalize) computed on host in numpy
    (0.01% of FLOPs); it determines the token->expert dispatch, i.e. the sharding.
  - Tensor-parallel over the FFN intermediate dim across 8 cores: core c owns
    I-slice [c*512:(c+1)*512) of every expert (ws rows for gate and up, w2s cols).
  - Top-2 sparsity: tokens are packed per expert EXACTLY (no padding): per
    expert, chunks of <=256 tokens (full 256-chunks plus one ragged tail).
  - All matmul operands are fp16 (1.0 PE cycles/row at any free size, half the
    HBM traffic of fp32); accumulation is fp32 in PSUM. Every matmul streams
    N=w tokens (gate/up: [128i x w], down: [128d x w]), so PE time is exactly
    proportional to routed tokens: no padding waste anywhere.
  - The combine weights are applied on the HOST during the final gather
    (out[t] = cw0[t]*packed[pos0[t]] + cw1[t]*packed[pos1[t]]), so the device
    writes unscaled partials and PSUM evacuation is a plain copy.
  - No on-device collective: each core writes its partial output in a
    d-major transposed layout [128, 16, ntok] (fp16); the host sums the 8
    partials, transposes, scales and gathers.
  - Pipelining: per chunk c the program issues P1(c) (gate/up+SwiGLU) then
    Down(c-1), so the PE never waits on the DVE/ACT producing h. Input loads
    (x chunks, weights) ride the SP sequencer queue with weights prefetched
    one expert ahead; output stores ride the ACT queue (issued after that
    chunk's evacuation copies), so they never stall input loads.
"""

import math

import numpy as np

T = 4096
D = 2048
E = 8
I = 4096
TOPK = 2
NCORES = 8
ISH = I // NCORES  # 512, per-core I shard
P = 128
DCH = D // P  # 16 d-chunks
ICH = ISH // P  # 4 i-chunks
CHUNK = 512  # max token chunk (PSUM bank holds 512 fp32 per partition)


def _host_router(x, router_w):
    """Replicate reference routing in numpy (fp32)."""
    logits = (x.astype(np.float64) @ router_w.astype(np.float64).T).astype(np.float32)
    m = logits.max(axis=-1, keepdims=True)
    ex = np.exp((logits - m).astype(np.float32))
    probs = ex / ex.sum(axis=-1, keepdims=True)
    # top-2, ties to lower index (matches jax.lax.top_k)
    top1 = probs.argmax(axis=-1)
    p = probs.copy()
    p[np.arange(T), top1] = -1.0
    top2 = p.argmax(axis=-1)
    w1 = probs[np.arange(T), top1]
    w2 = probs[np.arange(T), top2]
    s = w1 + w2
    return top1.astype(np.int64), top2.astype(np.int64), (w1 / s).astype(np.float32), (w2 / s).astype(np.float32)


_CACHE: dict = {}


def _build_bass(chunks):
    """Build the 8-core SPMD Bass program.

    chunks: tuple of (expert, width) in packed-token order; widths sum to T*TOPK.
    """
    import concourse.bacc as bacc
    import concourse.mybir as mybir
    import concourse.tile as tile

    f16 = mybir.dt.float16

    nchunks = len(chunks)
    xlen = sum(DCH * w for _, w in chunks)
    ntok = sum(w for _, w in chunks)

    # per-chunk packed-token base and x offset
    tokbase = []
    xoff = []
    tb = xo = 0
    for _, w in chunks:
        tokbase.append(tb)
        xoff.append(xo)
        tb += w
        xo += DCH * w

    # expert schedule: unique experts in chunk order, with local chunk counts
    experts_used = []
    for e, _ in chunks:
        if not experts_used or experts_used[-1] != e:
            experts_used.append(e)
    nch_of = {e: sum(1 for ee, _ in chunks if ee == e) for e in experts_used}
    ei_of_chunk = []  # expert-INDEX per global chunk
    loc_of_chunk = []  # local chunk index within its expert
    cur = -1
    loc = 0
    for e, _ in chunks:
        if cur == -1 or experts_used[cur] != e:
            cur += 1
            loc = 0
        ei_of_chunk.append(cur)
        loc_of_chunk.append(loc)
        loc += 1

    nc = bacc.Bacc("TRN2", target_bir_lowering=False)

    xtp_d = nc.dram_tensor("xtp", [P, xlen], f16, kind="ExternalInput")
    wst_d = nc.dram_tensor("wst", [E, DCH, P, 2 * ISH], f16, kind="ExternalInput")
    w2st_d = nc.dram_tensor("w2st", [E, ICH, P, D], f16, kind="ExternalInput")
    # partial output, d-major transposed: out[p, c, t] = partial[t, c*128 + p]
    out_d = nc.dram_tensor("out", [P, DCH, ntok], f16, kind="ExternalOutput")

    with tile.TileContext(nc) as tc:
        with (
            tc.tile_pool(name="wpool", bufs=32) as wpool,
            tc.tile_pool(name="w2pool", bufs=8) as w2pool,
            tc.tile_pool(name="xpool", bufs=3) as xpool,
            tc.tile_pool(name="spool", bufs=3) as spool,
            tc.tile_pool(name="hpool", bufs=2) as hpool,
            tc.tile_pool(name="opool", bufs=2) as opool,
            tc.tile_pool(name="misc", bufs=1) as mpool,
            tc.tile_pool(name="ph", bufs=5, space="PSUM") as ph_pool,
            tc.tile_pool(name="po", bufs=3, space="PSUM") as po_pool,
        ):
            wtiles: dict = {}
            w2tiles: dict = {}

            def wst_thunk(e, dc):
                def run():
                    t = wpool.tile([P, 2 * ISH], f16, tag="wst", name=f"wst{e}_{dc}")
                    nc.sync.dma_start(t[:], wst_d[e, dc])
                    wtiles.setdefault(e, {})[dc] = t

                return run

            def w2_thunk(e, ic):
                def run():
                    t = w2pool.tile([P, D], f16, tag="w2st", name=f"w2st{e}_{ic}")
                    nc.sync.dma_start(t[:], w2st_d[e, ic])
                    w2tiles.setdefault(e, {})[ic] = t

                return run

            # per expert-index: list of 20 weight-DMA thunks (wst dc0..15, w2 ic0..3)
            wthunks = [
                [wst_thunk(e, dc) for dc in range(DCH)]
                + [w2_thunk(e, ic) for ic in range(ICH)]
                for e in experts_used
            ]
            wissued = [0] * len(experts_used)

            def issue_weights(i, upto):
                upto = min(upto, len(wthunks[i]))
                while wissued[i] < upto:
                    wthunks[i][wissued[i]]()
                    wissued[i] += 1

            xts: dict = {}

            def issue_xt(g):
                e, w = chunks[g]
                t = xpool.tile([P, DCH * CHUNK], f16, tag="xt", name=f"xt{g}")
                nc.sync.dma_start(t[:, : DCH * w], xtp_d[:, xoff[g] : xoff[g] + DCH * w])
                xts[g] = t

            # PE warmup: dummy matmuls on a zeroed tile while the first DMAs
            # are in flight, so the PE clock is at full speed (the cost
            # model's p-state ramp needs ~3us of continuous PE activity) when
            # the first real matmul issues.
            wz = mpool.tile([P, 64], f16, name="wz")
            nc.vector.memset(wz[:], 0)
            pw = ph_pool.tile([P, CHUNK], mybir.dt.float32, tag="ph", name="pwarm")
            for k in range(70):
                nc.tensor.matmul(pw[:64, :64], wz[:], wz[:], start=True, stop=True)

            # startup: interleave per-dc slices of the first x chunk with the
            # first weight tiles so the PE's dc-outer sweep of chunk 0 starts
            # after ~2us of DMA and never starves thereafter.
            w0 = chunks[0][1]
            xt0 = xpool.tile([P, DCH * CHUNK], f16, tag="xt", name="xt0")
            xts[0] = xt0
            NPIECE = 6
            for dc in range(NPIECE):
                nc.sync.dma_start(
                    xt0[:, dc * w0 : (dc + 1) * w0],
                    xtp_d[:, dc * w0 : (dc + 1) * w0],
                )
                issue_weights(0, dc + 1)
            nc.sync.dma_start(
                xt0[:, NPIECE * w0 : DCH * w0], xtp_d[:, NPIECE * w0 : DCH * w0]
            )
            issue_weights(0, DCH)
            if nchunks > 1:
                issue_xt(1)
            issue_weights(0, DCH + ICH)

            hTs: dict = {}

            def phase1(g):
                e, w = chunks[g]
                xt = xts.pop(g)
                hT = hpool.tile([P, ICH * CHUNK], f16, tag="hT", name=f"hT{g}")
                for ic in range(ICH):
                    pg = ph_pool.tile([P, CHUNK], mybir.dt.float32, tag="ph", name=f"pg{g}_{ic}")
                    pu = ph_pool.tile([P, CHUNK], mybir.dt.float32, tag="ph", name=f"pu{g}_{ic}")
                    for dc in range(DCH):
                        wt = wtiles[e][dc]
                        xs = xt[:, dc * w : (dc + 1) * w]
                        nc.tensor.matmul(
                            pg[:, :w],
                            wt[:, ic * P : (ic + 1) * P],
                            xs,
                            start=(dc == 0),
                            stop=(dc == DCH - 1),
                        )
                        nc.tensor.matmul(
                            pu[:, :w],
                            wt[:, ISH + ic * P : ISH + (ic + 1) * P],
                            xs,
                            start=(dc == 0),
                            stop=(dc == DCH - 1),
                        )
                    sg = spool.tile([P, CHUNK], f16, tag="sg", name=f"sg{g}_{ic}")
                    nc.scalar.activation(
                        sg[:, :w], pg[:, :w], mybir.ActivationFunctionType.Silu
                    )
                    nc.vector.tensor_mul(
                        hT[:, ic * CHUNK : ic * CHUNK + w], sg[:, :w], pu[:, :w]
                    )
                hTs[g] = hT

            def phase1_dc_outer(g):
                # chunk-0 variant: all 8 accumulation groups live at once
                # (5 ph banks + 3 po banks) so the PE can consume each weight
                # tile the moment its DMA lands, with no ic-sweep re-reads.
                e, w = chunks[g]
                xt = xts.pop(g)
                hT = hpool.tile([P, ICH * CHUNK], f16, tag="hT", name=f"hT{g}")
                pgs = [
                    ph_pool.tile([P, CHUNK], mybir.dt.float32, tag="ph", name=f"pg{g}_{ic}")
                    for ic in range(ICH)
                ]
                pus = [ph_pool.tile([P, CHUNK], mybir.dt.float32, tag="ph", name=f"pu{g}_0")] + [
                    po_pool.tile([P, CHUNK], mybir.dt.float32, tag="po", name=f"pu{g}_{ic}")
                    for ic in range(1, ICH)
                ]
                for dc in range(DCH):
                    wt = wtiles[e][dc]
                    xs = xt[:, dc * w : (dc + 1) * w]
                    for ic in range(ICH):
                        nc.tensor.matmul(
                            pgs[ic][:, :w],
                            wt[:, ic * P : (ic + 1) * P],
                            xs,
                            start=(dc == 0),
                            stop=(dc == DCH - 1),
                        )
                        nc.tensor.matmul(
                            pus[ic][:, :w],
                            wt[:, ISH + ic * P : ISH + (ic + 1) * P],
                            xs,
                            start=(dc == 0),
                            stop=(dc == DCH - 1),
                        )
                for ic in range(ICH):
                    sg = spool.tile([P, CHUNK], f16, tag="sg", name=f"sg{g}_{ic}")
                    nc.scalar.activation(
                        sg[:, :w], pgs[ic][:, :w], mybir.ActivationFunctionType.Silu
                    )
                    nc.vector.tensor_mul(
                        hT[:, ic * CHUNK : ic * CHUNK + w], sg[:, :w], pus[ic][:, :w]
                    )
                hTs[g] = hT

            def down(g, fine_store=False):
                e, w = chunks[g]
                hT = hTs.pop(g)
                base = tokbase[g]
                osb = opool.tile([P, DCH, CHUNK], f16, tag="osb", name=f"osb{g}")
                for dc in range(DCH):
                    po = po_pool.tile([P, CHUNK], mybir.dt.float32, tag="po", name=f"po{g}_{dc}")
                    for ic in range(ICH):
                        nc.tensor.matmul(
                            po[:, :w],
                            w2tiles[e][ic][:, dc * P : (dc + 1) * P],
                            hT[:, ic * CHUNK : ic * CHUNK + w],
                            start=(ic == 0),
                            stop=(ic == ICH - 1),
                        )
                    # evacuate PSUM -> SBUF fp16 (plain copy; combine weights
                    # are applied on the host). Split ACT/DVE.
                    if dc % 2 == 0:
                        nc.scalar.activation(
                            osb[:, dc, :w], po[:, :w], mybir.ActivationFunctionType.Copy
                        )
                    else:
                        nc.vector.tensor_copy(osb[:, dc, :w], po[:, :w])
                    if fine_store:
                        # final chunk: store per-dc so the transfers overlap
                        # the remaining Down matmuls instead of sitting wholly
                        # in the program tail; alternate HWDGE (ACT) and
                        # SWDGE (gpsimd) so descriptor generation pipelines
                        eng = nc.scalar if dc % 2 == 0 else nc.gpsimd
                        eng.dma_start(out_d[:, dc, base : base + w], osb[:, dc, :w])
                if not fine_store:
                    # output store on the ACT queue (after its own evac
                    # copies): never blocks the SP input-load queue
                    nc.scalar.dma_start(
                        out_d[:, :, base : base + w], osb[:, :, :w]
                    )

            for g in range(nchunks):
                if g + 2 < nchunks:
                    issue_xt(g + 2)
                # prefetch next expert's weights, paced across this expert's chunks
                i = ei_of_chunk[g]
                if i + 1 < len(experts_used):
                    m = nch_of[experts_used[i]]
                    j = loc_of_chunk[g]
                    issue_weights(i + 1, math.ceil(20 * (j + 1) / m))
                if g == 0:
                    phase1_dc_outer(g)
                else:
                    phase1(g)
                if g > 0:
                    down(g - 1)
            down(nchunks - 1, fine_store=True)

    nc.compile()
    return nc


def _prepare(hidden_states, router_w, ws, w2s):
    """Host-side routing, packing, transposes, fp16 casts. Returns
    (chunks, ntok, pos, topw, shared inputs dict, per-core weight arrays)."""
    x = np.asarray(hidden_states, dtype=np.float32).reshape(T, D)
    router_w = np.asarray(router_w, dtype=np.float32)
    ws = np.asarray(ws, dtype=np.float32)
    w2s = np.asarray(w2s, dtype=np.float32)

    top1, top2, w1, w2 = _host_router(x, router_w)

    # per-expert token lists
    toks: list[list[int]] = [[] for _ in range(E)]
    for ti in (top1, top2):
        for t in range(T):
            toks[int(ti[t])].append(t)

    order = [e for e in range(E) if len(toks[e]) > 0]

    # exact packing: per expert, ceil(n/CHUNK) near-equal chunks (no tiny
    # tail chunks, whose fixed overheads would stall the PE)
    chunks: list[tuple[int, int]] = []
    perm: list[int] = []
    pos = np.zeros((TOPK, T), dtype=np.int64)
    seen: dict[int, int] = {}
    for e in order:
        n = len(toks[e])
        base = len(perm)
        for j, t in enumerate(toks[e]):
            k = seen.get(t, 0)
            pos[k, t] = base + j
            seen[t] = k + 1
        perm.extend(toks[e])
        parts = math.ceil(n / CHUNK)
        q, r = divmod(n, parts)
        chunks.extend([(e, q + 1)] * r)
        chunks.extend([(e, q)] * (parts - r))
    ntok = len(perm)
    perm_a = np.asarray(perm, dtype=np.int64)

    # NOTE: pos[k, t] maps (token, k-th choice) -> packed row, but the k-th
    # append pass order is (top1, top2); seen[] ensures pos rows follow that.
    topw = np.stack([w1, w2], axis=0)  # [2, T] renormalized weights

    # packed-transposed tokens, fp16, chunk-contiguous per partition:
    # per chunk (w tokens): xtp[p, off + dc*w + j] = x[perm[tb + j], dc*128 + p]
    xb = x[perm_a].astype(np.float16)  # [ntok, D]
    xlen = DCH * ntok
    xtp = np.empty((P, xlen), dtype=np.float16)
    tb = xo = 0
    for _, w in chunks:
        blk = xb[tb : tb + w].reshape(w, DCH, P).transpose(2, 1, 0).reshape(P, DCH * w)
        xtp[:, xo : xo + DCH * w] = blk
        tb += w
        xo += DCH * w

    # per-core weights (fp16)
    wst_all = []
    w2st_all = []
    gate = ws[:, :I, :]  # [E, I, D]
    up = ws[:, I:, :]
    for c in range(NCORES):
        lo, hi = c * ISH, (c + 1) * ISH
        # [E, DCH, P, 2*ISH]: [.., d-part, gate(ISH)||up(ISH)]
        g = gate[:, lo:hi, :].reshape(E, ISH, DCH, P).transpose(0, 2, 3, 1)
        u = up[:, lo:hi, :].reshape(E, ISH, DCH, P).transpose(0, 2, 3, 1)
        wst = np.concatenate([g, u], axis=3)
        wst_all.append(np.ascontiguousarray(wst, dtype=np.float16))
        # w2s[e] is [D, I]; lhsT tile [ic, p(i), d] = w2s[e, d, lo + ic*128 + p]
        w2t = w2s[:, :, lo:hi].transpose(0, 2, 1).reshape(E, ICH, P, D)
        w2st_all.append(np.ascontiguousarray(w2t, dtype=np.float16))

    shared = {"xtp": xtp}
    return tuple(chunks), ntok, pos, topw, shared, wst_all, w2st_all


def kernel(hidden_states, router_w, ws, w2s):
    from concourse import bass_utils

    hs = np.asarray(hidden_states)
    B, S, _ = hs.shape
    chunks, ntok, pos, topw, shared, wst_all, w2st_all = _prepare(
        hidden_states, router_w, ws, w2s
    )

    if chunks not in _CACHE:
        _CACHE[chunks] = _build_bass(chunks)
    nc = _CACHE[chunks]

    in_maps = [
        {**shared, "wst": wst_all[c], "w2st": w2st_all[c]} for c in range(NCORES)
    ]
    res = bass_utils.run_bass_kernel_spmd(nc, in_maps, core_ids=list(range(NCORES)))
    # host combine: sum the 8 I-shard partials (d-major transposed layout),
    # then scale by the combine weights and gather the two expert
    # contributions per token
    acc = np.zeros((P, DCH, ntok), dtype=np.float32)
    for c in range(NCORES):
        acc += res.results[c]["out"].astype(np.float32)
    packed = acc.transpose(2, 1, 0).reshape(ntok, D)  # [t, dc*128+p]
    out = topw[0][:, None] * packed[pos[0]] + topw[1][:, None] * packed[pos[1]]
    return out.reshape(B, S, D).astype(np.float32)
